# revision 1
# baseline (speedup 1.0000x reference)
"""Trainium2 Bass kernel for DeepEdgeConvolution (gnn_message_passing).

Math (reference):
    bei = edge_nodes[:, src] + edge_nodes[:, dst]          # [B, E]
    bei = bei / row_sum (0 if empty row)
    h = BN1(relu(x @ W0 + b0)); h = BN2(relu(h @ W1 + b1)); h = h @ W2 + b2
    out = bei @ h                                          # [B, K]

Restructured: fold BN1 into (W1, b1) and BN2 into (W2, b2):
    a1 = relu(x @ W0 + b0)             (BN1 stats over E -> s1, t1)
    W1' = diag(s1) W1 ; b1' = t1 @ W1 + b1
    a2 = relu(a1 @ W1' + b1')          (BN2 stats over E -> s2, t2)
    W2' = diag(s2) W2 ; b2' = t2 @ W2 + b2
    out = diag(inv) [ (bei_raw @ a2) @ W2' + row_sum_raw x b2' ]
So the [B,E]x[E,K] spmm collapses to G = bei_raw @ a2 ([B, H]) accumulated in
PSUM over edge tiles via augmented matmuls:
    [bei | 1].T @ [a2 | 1] -> [[G, rs], [sum2, cnt]]   (one PSUM chain)
    [bei | 1].T @ a2^2     -> [.., sumsq2]             (second PSUM chain)
followed by a tiny epilogue after an AllReduce of [G | rs | BN2 sums].

Sharding: edges across 8 cores; two streaming passes over x^T per core
(pass A: BN1 stats via bn_stats; pass B: recompute a1, then a2, G). Edge
features are passed pre-transposed (x^T), packed two 512-edge tiles across
the 128 SBUF partitions (D=64). bei tiles come from batched indirect-DMA
gathers of rows of edge_nodes^T (one 32-float row per edge endpoint).
"""

import numpy as np

import concourse.bacc as bacc
import concourse.bass as bass
import concourse.tile as tile
from concourse import mybir
from concourse.bass_utils import run_bass_kernel_spmd
from concourse.masks import make_identity

f32 = mybir.dt.float32
bf16 = mybir.dt.bfloat16
i32 = mybir.dt.int32

NCORES = 8
B, D, H, KDIM = 32, 64, 128, 128
EPS = 1e-5
TILE = 512           # edges per tile
SUB = 128            # edges per matmul subtile
GATHER_BATCH = 2048  # edges per indirect-DMA gather call (16 subtiles)

# compute dtype: "f32" (exact-ish) or "bf16" (fast). Chosen empirically.
COMPUTE_DT = "f32"


def _np_dt(dt):
    if dt == bf16:
        import ml_dtypes
        return ml_dtypes.bfloat16
    return np.float32


def build_nc(ESH, N, E_total, dt_c=f32, dt_en=f32, debug=False):
    """Build the SPMD Bass program. ESH = padded edges per core."""
    assert ESH % GATHER_BATCH == 0
    NT = ESH // TILE          # tiles per core
    NTP = NT // 2             # tile pairs (xTi packs 2 tiles across 128 parts)
    NSUB = ESH // SUB         # 128-edge subtiles per core
    n_pad = ESH - E_total // NCORES
    assert E_total % NCORES == 0
    NS = TILE // SUB          # subtiles per tile (4)

    nc = bass.Bass()

    # ---- I/O ----
    xTi = nc.dram_tensor("xTi", [128, ESH // 2], dt_c, kind="ExternalInput")
    enT = nc.dram_tensor("enT", [N + 1, B], dt_en, kind="ExternalInput")
    srcT = nc.dram_tensor("srcT", [128, NSUB], i32, kind="ExternalInput")
    dstT = nc.dram_tensor("dstT", [128, NSUB], i32, kind="ExternalInput")
    W0d = nc.dram_tensor("W0", [2 * D, H], dt_c, kind="ExternalInput")
    W1d = nc.dram_tensor("W1", [H, H], f32, kind="ExternalInput")
    W2d = nc.dram_tensor("W2", [H, KDIM], f32, kind="ExternalInput")
    b0cd = nc.dram_tensor("b0c", [H, 1], f32, kind="ExternalInput")
    b1rd = nc.dram_tensor("b1r", [1, H], f32, kind="ExternalInput")
    b1cd = nc.dram_tensor("b1c", [H, 1], f32, kind="ExternalInput")
    b2rd = nc.dram_tensor("b2r", [1, KDIM], f32, kind="ExternalInput")
    g0cd = nc.dram_tensor("g0c", [H, 1], f32, kind="ExternalInput")
    bt0cd = nc.dram_tensor("bt0c", [H, 1], f32, kind="ExternalInput")
    g1cd = nc.dram_tensor("g1c", [H, 1], f32, kind="ExternalInput")
    bt1cd = nc.dram_tensor("bt1c", [H, 1], f32, kind="ExternalInput")
    outd = nc.dram_tensor("out", [B, KDIM], f32, kind="ExternalOutput")
    if debug:
        dbg_g = nc.dram_tensor("dbg_g", [B + 1, H + 1], f32, kind="ExternalOutput")
        dbg_ss = nc.dram_tensor("dbg_ss", [1, H], f32, kind="ExternalOutput")
        dbg_garr = nc.dram_tensor("dbg_garr", [H, H + 3], f32, kind="ExternalOutput")
        dbg_gall = nc.dram_tensor("dbg_gall", [H, H + 3], f32, kind="ExternalOutput")
        dbg_s2t2 = nc.dram_tensor("dbg_s2t2", [H, 4], f32, kind="ExternalOutput")
        dbg_fin = nc.dram_tensor("dbg_fin", [B, KDIM + 3], f32, kind="ExternalOutput")

    rg = [list(range(NCORES))]

    with tile.TileContext(nc) as tc:
        with (
            tc.tile_pool(name="const", bufs=1) as cpool,
            tc.tile_pool(name="xp", bufs=6) as xpool,
            tc.tile_pool(name="a1p", bufs=12) as a1pool,
            tc.tile_pool(name="gat", bufs=10) as gpool,
            tc.tile_pool(name="beip", bufs=8) as bpool,
            tc.tile_pool(name="zbp", bufs=3) as zbpool,
            tc.tile_pool(name="a2p", bufs=3) as a2pool,
            tc.tile_pool(name="sqp", bufs=3) as sqpool,
            tc.tile_pool(name="misc", bufs=2) as mpool,
            tc.tile_pool(name="psA", bufs=2, space="PSUM") as psA,
            tc.tile_pool(name="psB", bufs=2, space="PSUM") as psB,
            tc.tile_pool(name="psG", bufs=1, space="PSUM") as psG,
            tc.tile_pool(name="psS", bufs=2, space="PSUM") as psS,
            tc.tile_pool(name="dram", bufs=1, space="DRAM") as dpool,
        ):
            # ---- constants / params in SBUF ----
            w0sb = cpool.tile([128, H], dt_c)  # W0 duplicated on both halves
            nc.sync.dma_start(w0sb[:], W0d[:])
            w1sb = cpool.tile([H, H], f32)
            nc.sync.dma_start(w1sb[:], W1d[:])
            w2sb = cpool.tile([H, KDIM], f32)
            nc.sync.dma_start(w2sb[:], W2d[:])
            b0c = cpool.tile([H, 1], f32)
            nc.sync.dma_start(b0c[:], b0cd[:])
            b1r = cpool.tile([1, H], f32)
            nc.sync.dma_start(b1r[:], b1rd[:])
            b1c = cpool.tile([H, 1], f32)
            nc.sync.dma_start(b1c[:], b1cd[:])
            b2r = cpool.tile([1, KDIM], f32)
            nc.sync.dma_start(b2r[:], b2rd[:])
            g0c = cpool.tile([H, 1], f32)
            nc.sync.dma_start(g0c[:], g0cd[:])
            bt0c = cpool.tile([H, 1], f32)
            nc.sync.dma_start(bt0c[:], bt0cd[:])
            g1c = cpool.tile([H, 1], f32)
            nc.sync.dma_start(g1c[:], g1cd[:])
            bt1c = cpool.tile([H, 1], f32)
            nc.sync.dma_start(bt1c[:], bt1cd[:])
            srcsb = cpool.tile([128, NSUB], i32)
            nc.sync.dma_start(srcsb[:], srcT[:])
            dstsb = cpool.tile([128, NSUB], i32)
            nc.sync.dma_start(dstsb[:], dstT[:])

            ones_row = cpool.tile([1, H], f32)
            nc.vector.memset(ones_row[:], 1.0)
            id32 = cpool.tile([B, B], f32)
            make_identity(nc, id32[:])
            id33 = cpool.tile([B + 1, B + 1], f32)
            make_identity(nc, id33[:])

            stats1 = cpool.tile([H, 6 * NT], f32)

            # pad-row constants: a1_pad = relu(b0) in compute dtype
            a1_pad_c = cpool.tile([H, 1], dt_c)
            nc.scalar.activation(a1_pad_c[:], b0c[:], mybir.ActivationFunctionType.Relu)
            a1_pad = cpool.tile([H, 1], f32)
            nc.vector.tensor_copy(a1_pad[:], a1_pad_c[:])
            a1_pad_sq = cpool.tile([H, 1], f32)
            nc.vector.tensor_mul(a1_pad_sq[:], a1_pad[:], a1_pad[:])

            # ================= PASS A: BN1 stats =================
            for tp in range(NTP):
                xti = xpool.tile([128, TILE], dt_c, tag="xti")
                nc.sync.dma_start(xti[:], xTi[:, tp * TILE:(tp + 1) * TILE])
                for u in range(2):
                    t = 2 * tp + u
                    z1 = psA.tile([H, TILE], f32, space="PSUM", tag="z1")
                    nc.tensor.matmul(
                        z1[:], lhsT=w0sb[u * D:(u + 1) * D, :],
                        rhs=xti[u * D:(u + 1) * D, :], start=True, stop=True)
                    a1 = a1pool.tile([H, TILE], dt_c, tag="a1")
                    nc.scalar.activation(
                        a1[:], z1[:], mybir.ActivationFunctionType.Relu, bias=b0c[:, 0:1])
                    nc.vector.bn_stats(stats1[:, 6 * t:6 * t + 6], a1[:])

            # ---- AllReduce #1: BN1 sums ----
            mv1 = mpool.tile([H, 2], f32, tag="mv")
            nc.vector.bn_aggr(mv1[:], stats1[:])
            # sums with pad correction
            ar1 = mpool.tile([H, 2], f32, tag="ar")
            tmp_a = mpool.tile([H, 1], f32, tag="tmpa")
            tmp_b = mpool.tile([H, 1], f32, tag="tmpb")
            # sum_raw = mean * ESH ; corrected -= n_pad * a1_pad
            nc.scalar.mul(tmp_a[:], a1_pad[:], float(n_pad))
            nc.scalar.mul(tmp_b[:], mv1[:, 0:1], float(ESH))
            nc.vector.tensor_sub(ar1[:, 0:1], tmp_b[:], tmp_a[:])
            # ss_raw = (var + mean^2) * ESH ; corrected -= n_pad * a1_pad^2
            msq1 = mpool.tile([H, 1], f32, tag="msq")
            nc.vector.tensor_mul(msq1[:], mv1[:, 0:1], mv1[:, 0:1])
            nc.vector.tensor_add(msq1[:], msq1[:], mv1[:, 1:2])
            nc.scalar.mul(tmp_b[:], msq1[:], float(ESH))
            nc.scalar.mul(tmp_a[:], a1_pad_sq[:], float(n_pad))
            nc.vector.tensor_sub(ar1[:, 1:2], tmp_b[:], tmp_a[:])

            cc1_in = dpool.tile([H, 2], f32)
            cc1_out = dpool.tile([H, 2], f32)
            nc.sync.dma_start(cc1_in[:], ar1[:])
            nc.gpsimd.collective_compute(
                "AllReduce", mybir.AluOpType.add, replica_groups=rg,
                ins=[cc1_in.opt()], outs=[cc1_out.opt()])
            gs1 = mpool.tile([H, 2], f32, tag="gs")
            nc.sync.dma_start(gs1[:], cc1_out[:])

            # mu, var, s1, t1
            mu1 = mpool.tile([H, 1], f32, tag="mu")
            nc.scalar.mul(mu1[:], gs1[:, 0:1], 1.0 / E_total)
            ex2 = mpool.tile([H, 1], f32, tag="ex2")
            nc.scalar.mul(ex2[:], gs1[:, 1:2], 1.0 / E_total)
            var1 = mpool.tile([H, 1], f32, tag="var")
            nc.vector.tensor_mul(var1[:], mu1[:], mu1[:])
            nc.vector.tensor_sub(var1[:], ex2[:], var1[:])
            sd1 = mpool.tile([H, 1], f32, tag="sd")
            nc.vector.tensor_scalar_add(sd1[:], var1[:], EPS)
            nc.scalar.sqrt(sd1[:], sd1[:])
            isd1 = mpool.tile([H, 1], f32, tag="isd")
            nc.vector.reciprocal(isd1[:], sd1[:])
            s1 = mpool.tile([H, 1], f32, tag="s1")
            nc.vector.tensor_mul(s1[:], g0c[:], isd1[:])
            t1 = mpool.tile([H, 1], f32, tag="t1")
            nc.vector.tensor_mul(t1[:], mu1[:], s1[:])
            nc.vector.tensor_sub(t1[:], bt0c[:], t1[:])

            # W1' (compute dtype), b1' broadcast [H, TILE], b1' col, a2_pad
            w1p = cpool.tile([H, H], dt_c)
            nc.vector.tensor_scalar_mul(w1p[:], w1sb[:], s1[:, 0:1])
            pr = psS.tile([1, H], f32, space="PSUM", tag="pss")
            nc.tensor.matmul(pr[:], lhsT=t1[:], rhs=w1sb[:], start=True, stop=True)
            b1p_row = mpool.tile([1, H], f32, tag="b1pr")
            nc.vector.tensor_add(b1p_row[:], pr[:], b1r[:])
            bc_ps = psS.tile([H, H], f32, space="PSUM", tag="pss")
            nc.tensor.matmul(bc_ps[:], lhsT=ones_row[:], rhs=b1p_row[:], start=True, stop=True)
            b1bc = cpool.tile([H, TILE], f32)
            for s in range(TILE // H):
                nc.vector.tensor_copy(b1bc[:, s * H:(s + 1) * H], bc_ps[:])
            pc = psS.tile([H, 1], f32, space="PSUM", tag="pss")
            nc.tensor.matmul(pc[:], lhsT=w1sb[:], rhs=t1[:], start=True, stop=True)
            b1p_col = mpool.tile([H, 1], f32, tag="b1pc")
            nc.vector.tensor_add(b1p_col[:], pc[:], b1c[:])
            pap = psS.tile([H, 1], f32, space="PSUM", tag="pss")
            nc.tensor.matmul(pap[:], lhsT=w1p[:], rhs=a1_pad_c[:], start=True, stop=True)
            a2_pad_c = cpool.tile([H, 1], dt_c)
            nc.scalar.activation(
                a2_pad_c[:], pap[:], mybir.ActivationFunctionType.Relu, bias=b1p_col[:, 0:1])
            a2_pad = cpool.tile([H, 1], f32)
            nc.vector.tensor_copy(a2_pad[:], a2_pad_c[:])
            a2_pad_sq = cpool.tile([H, 1], f32)
            nc.vector.tensor_mul(a2_pad_sq[:], a2_pad[:], a2_pad[:])

            # ============ PASS B: a2, BN2 sums, G accumulation ============
            # gacc1: [bei|1].T @ [a2|1] -> [0:32,0:H]=G, [0:32,H]=rs,
            #                             [32,0:H]=sum2, [32,H]=count
            # gacc2: [bei|1].T @ a2^2   -> [32,0:H]=sumsq2 (rows 0:32 unused)
            gacc1 = psG.tile([B + 1, H + 1], f32, space="PSUM", tag="gacc1")
            gacc2 = psG.tile([B + 1, H], f32, space="PSUM", tag="gacc2")
            nsl = GATHER_BATCH // SUB  # 16 subtile slots per batch
            bei = None
            for tp in range(NTP):
                if tp % 2 == 0:
                    gb = tp // 2
                    # per-subtile [128,1]-offset gathers: the only indirect-DMA
                    # form HW SWDGE supports (one offset per partition)
                    sg = gpool.tile([128, nsl * B], dt_en, tag="sg")
                    dg = gpool.tile([128, nsl * B], dt_en, tag="dg")
                    for jj in range(nsl):
                        nc.gpsimd.indirect_dma_start(
                            out=sg[:, jj * B:(jj + 1) * B], out_offset=None, in_=enT[:],
                            in_offset=bass.IndirectOffsetOnAxis(
                                ap=srcsb[:, gb * nsl + jj:gb * nsl + jj + 1], axis=0))
                        nc.gpsimd.indirect_dma_start(
                            out=dg[:, jj * B:(jj + 1) * B], out_offset=None, in_=enT[:],
                            in_offset=bass.IndirectOffsetOnAxis(
                                ap=dstsb[:, gb * nsl + jj:gb * nsl + jj + 1], axis=0))
                    # bei33: per subtile [bei (32 cols) | ones (1 col)]
                    bei = bpool.tile([128, nsl * (B + 1)], dt_c, tag="bei")
                    bei3d = bei[:].rearrange("p (g c) -> p g c", c=B + 1)
                    nc.vector.tensor_add(
                        bei3d[:, :, 0:B],
                        sg[:].rearrange("p (g c) -> p g c", c=B),
                        dg[:].rearrange("p (g c) -> p g c", c=B))
                    nc.vector.memset(bei3d[:, :, B:B + 1], 1.0)

                xti = xpool.tile([128, TILE], dt_c, tag="xti")
                nc.sync.dma_start(xti[:], xTi[:, tp * TILE:(tp + 1) * TILE])
                for u in range(2):
                    t = 2 * tp + u
                    z1 = psA.tile([H, TILE], f32, space="PSUM", tag="z1")
                    nc.tensor.matmul(
                        z1[:], lhsT=w0sb[u * D:(u + 1) * D, :],
                        rhs=xti[u * D:(u + 1) * D, :], start=True, stop=True)
                    a1 = a1pool.tile([H, TILE], dt_c, tag="a1")
                    nc.scalar.activation(
                        a1[:], z1[:], mybir.ActivationFunctionType.Relu, bias=b0c[:, 0:1])
                    z2 = psB.tile([H, TILE], f32, space="PSUM", tag="z2")
                    for s in range(NS):
                        nc.tensor.matmul(
                            z2[:, s * H:(s + 1) * H], lhsT=a1[:, s * SUB:(s + 1) * SUB],
                            rhs=w1p[:], start=True, stop=True)
                    z2b = zbpool.tile([H, TILE], f32, tag="z2b")
                    nc.vector.tensor_add(z2b[:], z2[:], b1bc[:])
                    # a2aug: per subtile [a2 (128 cols) | ones (1 col)]
                    a2 = a2pool.tile([H, NS * (H + 1)], dt_c, tag="a2")
                    a23d = a2[:].rearrange("p (g c) -> p g c", c=H + 1)
                    nc.scalar.activation(
                        a23d[:, :, 0:H], z2b[:].rearrange("p (g c) -> p g c", c=H),
                        mybir.ActivationFunctionType.Relu)
                    nc.vector.memset(a23d[:, :, H:H + 1], 1.0)
                    sq = sqpool.tile([H, TILE], dt_c, tag="sq")
                    sq3d = sq[:].rearrange("p (g c) -> p g c", c=H)
                    nc.vector.tensor_mul(sq3d[:], a23d[:, :, 0:H], a23d[:, :, 0:H])
                    first = (t == 0)
                    last = (t == NT - 1)
                    for s in range(NS):
                        jj = (t % 4) * NS + s
                        bei_sl = bei[:, jj * (B + 1):(jj + 1) * (B + 1)]
                        nc.tensor.matmul(
                            gacc1[:], lhsT=bei_sl, rhs=a2[:, s * (H + 1):(s + 1) * (H + 1)],
                            start=(first and s == 0), stop=(last and s == NS - 1),
                            skip_group_check=True)
                        nc.tensor.matmul(
                            gacc2[:], lhsT=bei_sl, rhs=sq[:, s * H:(s + 1) * H],
                            start=(first and s == 0), stop=(last and s == NS - 1),
                            skip_group_check=True)

            # ---- AllReduce #2: [G | rs | BN2 sums] ----
            gsb1 = mpool.tile([B + 1, H + 1], f32, tag="gsb1")
            nc.vector.tensor_copy(gsb1[:], gacc1[:])
            gsb2 = mpool.tile([B + 1, H], f32, tag="gsb2")
            nc.vector.tensor_copy(gsb2[:], gacc2[:])
            if debug:
                nc.sync.dma_start(dbg_g[:], gsb1[:])
                nc.sync.dma_start(dbg_ss[:], gsb2[B:B + 1, :])
            # transpose [33, H] blocks via identity matmuls; col 32 = the sums
            tr1_ps = psS.tile([H, B + 1], f32, space="PSUM", tag="pss")
            nc.tensor.matmul(tr1_ps[:], lhsT=gsb1[:, 0:H], rhs=id33[:],
                             start=True, stop=True)
            tr2_ps = psS.tile([H, B + 1], f32, space="PSUM", tag="pss")
            nc.tensor.matmul(tr2_ps[:], lhsT=gsb2[:, 0:H], rhs=id33[:],
                             start=True, stop=True)

            garr = mpool.tile([H, H + 3], f32, tag="garr")
            nc.vector.memset(garr[:], 0.0)
            nc.vector.tensor_copy(garr[0:B, 0:H + 1], gsb1[0:B, :])
            # sum2/ss2 with pad corrections
            nc.scalar.mul(tmp_a[:], a2_pad[:], float(n_pad))
            nc.vector.tensor_sub(garr[:, H + 1:H + 2], tr1_ps[:, B:B + 1], tmp_a[:])
            nc.scalar.mul(tmp_a[:], a2_pad_sq[:], float(n_pad))
            nc.vector.tensor_sub(garr[:, H + 2:H + 3], tr2_ps[:, B:B + 1], tmp_a[:])

            if debug:
                nc.sync.dma_start(dbg_garr[:], garr[:])
            cc2_in = dpool.tile([H, H + 3], f32)
            cc2_out = dpool.tile([H, H + 3], f32)
            nc.sync.dma_start(cc2_in[:], garr[:])
            nc.gpsimd.collective_compute(
                "AllReduce", mybir.AluOpType.add, replica_groups=rg,
                ins=[cc2_in.opt()], outs=[cc2_out.opt()])
            gall = mpool.tile([H, H + 3], f32, tag="gall")
            nc.sync.dma_start(gall[:], cc2_out[:])
            if debug:
                nc.sync.dma_start(dbg_gall[:], gall[:])

            # ---- epilogue ----
            mu2 = mpool.tile([H, 1], f32, tag="mu")
            nc.scalar.mul(mu2[:], gall[:, H + 1:H + 2], 1.0 / E_total)
            ex2b = mpool.tile([H, 1], f32, tag="ex2")
            nc.scalar.mul(ex2b[:], gall[:, H + 2:H + 3], 1.0 / E_total)
            var2 = mpool.tile([H, 1], f32, tag="var")
            nc.vector.tensor_mul(var2[:], mu2[:], mu2[:])
            nc.vector.tensor_sub(var2[:], ex2b[:], var2[:])
            sd2 = mpool.tile([H, 1], f32, tag="sd")
            nc.vector.tensor_scalar_add(sd2[:], var2[:], EPS)
            nc.scalar.sqrt(sd2[:], sd2[:])
            isd2 = mpool.tile([H, 1], f32, tag="isd")
            nc.vector.reciprocal(isd2[:], sd2[:])
            s2 = mpool.tile([H, 1], f32, tag="s1")
            nc.vector.tensor_mul(s2[:], g1c[:], isd2[:])
            t2 = mpool.tile([H, 1], f32, tag="t1")
            nc.vector.tensor_mul(t2[:], mu2[:], s2[:])
            nc.vector.tensor_sub(t2[:], bt1c[:], t2[:])

            if debug:
                dbgs2 = mpool.tile([H, 4], f32, tag="dbgs2")
                nc.vector.tensor_copy(dbgs2[:, 0:1], s2[:])
                nc.vector.tensor_copy(dbgs2[:, 1:2], t2[:])
                nc.vector.tensor_copy(dbgs2[:, 2:3], mu2[:])
                nc.vector.tensor_copy(dbgs2[:, 3:4], var2[:])
                nc.sync.dma_start(dbg_s2t2[:], dbgs2[:])
            w2p = mpool.tile([H, KDIM], f32, tag="w2p")
            nc.vector.tensor_scalar_mul(w2p[:], w2sb[:], s2[:, 0:1])
            pr2 = psS.tile([1, KDIM], f32, space="PSUM", tag="pss")
            nc.tensor.matmul(pr2[:], lhsT=t2[:], rhs=w2sb[:], start=True, stop=True)
            b2p_row = mpool.tile([1, KDIM], f32, tag="b1pr")
            nc.vector.tensor_add(b2p_row[:], pr2[:], b2r[:])

            # inv / mask from rs = gall[0:B, H]
            rs = mpool.tile([B, 1], f32, tag="rs")
            nc.vector.tensor_copy(rs[:], gall[0:B, H:H + 1])
            mask = mpool.tile([B, 1], f32, tag="mask")
            nc.scalar.sign(mask[:], rs[:])
            om = mpool.tile([B, 1], f32, tag="om")
            nc.scalar.mul(om[:], mask[:], -1.0)
            nc.vector.tensor_scalar_add(om[:], om[:], 1.0)
            safe = mpool.tile([B, 1], f32, tag="safe")
            nc.vector.tensor_add(safe[:], rs[:], om[:])
            inv = mpool.tile([B, 1], f32, tag="inv")
            nc.vector.reciprocal(inv[:], safe[:])
            nc.vector.tensor_mul(inv[:], inv[:], mask[:])

            # G^T via matmul with identity; mask row likewise
            gt_ps = psS.tile([H, B], f32, space="PSUM", tag="pss")
            nc.tensor.matmul(gt_ps[:], lhsT=gall[0:B, 0:H], rhs=id32[:], start=True, stop=True)
            gt_sb = mpool.tile([H, B], f32, tag="gt")
            nc.vector.tensor_copy(gt_sb[:], gt_ps[:])
            mr_ps = psS.tile([1, B], f32, space="PSUM", tag="pss")
            nc.tensor.matmul(mr_ps[:], lhsT=rs[:], rhs=id32[:], start=True, stop=True)
            mr_sb = mpool.tile([1, B], f32, tag="mr")
            nc.vector.tensor_copy(mr_sb[:], mr_ps[:])

            out_ps = psS.tile([B, KDIM], f32, space="PSUM", tag="pss")
            nc.tensor.matmul(out_ps[:], lhsT=gt_sb[:], rhs=w2p[:], start=True, stop=False)
            nc.tensor.matmul(out_ps[:], lhsT=mr_sb[:], rhs=b2p_row[:], start=False, stop=True)
            outsb = mpool.tile([B, KDIM], f32, tag="outsb")
            nc.vector.tensor_scalar_mul(outsb[:], out_ps[:], inv[:, 0:1])
            nc.sync.dma_start(outd[:], outsb[:])
            if debug:
                dbgf = mpool.tile([B, KDIM + 3], f32, tag="dbgf")
                nc.vector.tensor_copy(dbgf[:, 0:KDIM], out_ps[:])
                nc.vector.tensor_copy(dbgf[:, KDIM:KDIM + 1], rs[:])
                nc.vector.tensor_copy(dbgf[:, KDIM + 1:KDIM + 2], mask[:])
                nc.vector.tensor_copy(dbgf[:, KDIM + 2:KDIM + 3], inv[:])
                nc.sync.dma_start(dbg_fin[:], dbgf[:])

    # Legalize waits for walrus (TRN2: max 1 wait/instruction; extras are
    # spilled onto ldweights / event-semaphore instructions).
    import bass_rust as _br
    _br.move_matmul_waits_to_ldweights(nc.m)
    _br.generate_event_semaphores(nc)
    nc.finalize()
    return nc


def _ceil_to(x, m):
    return (x + m - 1) // m * m


def make_inputs(inputs, ESH, N, dt_c=f32, dt_en=f32):
    """Host-side shard/layout prep. Returns in_maps for run_bass_kernel_spmd."""
    np_c = _np_dt(dt_c)
    np_en = _np_dt(dt_en)
    en = np.asarray(inputs["edge_nodes"], dtype=np.float32)
    x = np.asarray(inputs["edge_feats"], dtype=np.float32)
    src = np.asarray(inputs["src"]).astype(np.int32)
    dst = np.asarray(inputs["dst"]).astype(np.int32)
    E = x.shape[0]
    esh_real = E // NCORES

    enT = np.zeros((N + 1, B), dtype=np_en)
    enT[:N] = en.T.astype(np_en)

    common = dict(
        enT=enT,
        W0=np.vstack([np.asarray(inputs["W0"], np.float32)] * 2).astype(np_c),
        W1=np.asarray(inputs["W1"], np.float32),
        W2=np.asarray(inputs["W2"], np.float32),
        b0c=np.asarray(inputs["b0"], np.float32).reshape(H, 1),
        b1r=np.asarray(inputs["b1"], np.float32).reshape(1, H),
        b1c=np.asarray(inputs["b1"], np.float32).reshape(H, 1),
        b2r=np.asarray(inputs["b2"], np.float32).reshape(1, KDIM),
        g0c=np.asarray(inputs["g0"], np.float32).reshape(H, 1),
        bt0c=np.asarray(inputs["bt0"], np.float32).reshape(H, 1),
        g1c=np.asarray(inputs["g1"], np.float32).reshape(H, 1),
        bt1c=np.asarray(inputs["bt1"], np.float32).reshape(H, 1),
    )

    in_maps = []
    for c in range(NCORES):
        lo = c * esh_real
        xs = x[lo:lo + esh_real]
        xT = np.zeros((D, ESH), np.float32)
        xT[:, :esh_real] = xs.T
        NTP = ESH // (2 * TILE)
        xTi = np.ascontiguousarray(
            xT.reshape(D, NTP, 2, TILE).transpose(2, 0, 1, 3).reshape(128, ESH // 2)
        ).astype(np_c)
        srcs = np.full(ESH, N, np.int32)
        srcs[:esh_real] = src[lo:lo + esh_real]
        dsts = np.full(ESH, N, np.int32)
        dsts[:esh_real] = dst[lo:lo + esh_real]
        srcTn = np.ascontiguousarray(srcs.reshape(ESH // 128, 128).T)
        dstTn = np.ascontiguousarray(dsts.reshape(ESH // 128, 128).T)
        in_maps.append(dict(common, xTi=xTi, srcT=srcTn, dstT=dstTn))
    return in_maps


_NC_CACHE = {}


def kernel(**inputs):
    dt_c = bf16 if COMPUTE_DT == "bf16" else f32
    dt_en = dt_c
    x = np.asarray(inputs["edge_feats"])
    en = np.asarray(inputs["edge_nodes"])
    E = x.shape[0]
    N = en.shape[1]
    ESH = _ceil_to(E // NCORES, GATHER_BATCH)
    key = (ESH, N, E, COMPUTE_DT)
    if key not in _NC_CACHE:
        _NC_CACHE[key] = build_nc(ESH, N, E, dt_c=dt_c, dt_en=dt_en)
    nc = _NC_CACHE[key]
    in_maps = make_inputs(inputs, ESH, N, dt_c=dt_c, dt_en=dt_en)
    res = run_bass_kernel_spmd(nc, in_maps, list(range(NCORES)))
    return np.asarray(res.results[0]["out"], np.float32)



# revision 5
# speedup vs baseline: 1.0689x; 1.0689x over previous
"""Trainium2 Bass kernel for DeepEdgeConvolution (gnn_message_passing).

Math (reference):
    bei = edge_nodes[:, src] + edge_nodes[:, dst]          # [B, E]
    bei = bei / row_sum (0 if empty row)
    h = BN1(relu(x @ W0 + b0)); h = BN2(relu(h @ W1 + b1)); h = h @ W2 + b2
    out = bei @ h                                          # [B, K]

Restructured: fold BN1 into (W1, b1) and BN2 into (W2, b2):
    a1 = relu(x @ W0 + b0)             (BN1 stats over E -> s1, t1)
    W1' = diag(s1) W1 ; b1' = t1 @ W1 + b1
    a2 = relu(a1 @ W1' + b1')          (BN2 stats over E -> s2, t2)
    W2' = diag(s2) W2 ; b2' = t2 @ W2 + b2
    out = diag(inv) [ (bei_raw @ a2) @ W2' + rs_raw x b2' ]

Sharding: edges across 8 cores; two streaming passes over x^T per core
(pass A: BN1 stats via bn_stats; pass B: recompute a1, then a2, G).

bei is computed on the HOST (sharding the columns of batch_edge_idcs per the
sharding hint): beiT is streamed as a dense packed input [128, NSUB*33] where
each 33-col block is [bei | 1] for one 128-edge subtile (edges on partitions).
Pad edges get all-zero columns (including the ones entry), which kills every
pad correction. Row sums rs / inv are exact small host-side inputs.

G accumulation (the [B,E]x[E,K] spmm): per 128-edge subtile,
    psG += a2_sub^T @ [bei_aug | a2_sub]    -> [H, 33 + H]
giving G^T (cols 0:32), sum(a2) (col 32, via the ones column) and the a2 Gram
matrix whose diagonal is sum(a2^2) -- one PSUM chain yields everything BN2 and
the final matmul need.  One AllReduce of [H, 34] follows; the epilogue is a
couple of tiny matmuls.
"""

import numpy as np

import concourse.bacc as bacc
import concourse.bass as bass
import concourse.tile as tile
from concourse import mybir
from concourse.bass_utils import run_bass_kernel_spmd
from concourse.masks import make_identity

f32 = mybir.dt.float32
bf16 = mybir.dt.bfloat16
i32 = mybir.dt.int32

NCORES = 8
B, D, H, KDIM = 32, 64, 128, 128
EPS = 1e-5
TILE = 512           # edges per tile
SUB = 128            # edges per matmul subtile
GATHER_BATCH = 2048  # edges per DMA chunk (ESH must be a multiple)

# compute dtype: "f32" (exact-ish) or "bf16" (fast).
COMPUTE_DT = "bf16"

BW = B + 1           # bei block width: [bei (32) | ones (1)]


def _np_dt(dt):
    if dt == bf16:
        import ml_dtypes
        return ml_dtypes.bfloat16
    return np.float32


def build_nc(ESH, N, E_total, dt_c=bf16, dt_en=None, debug=False):
    """Build the SPMD Bass program. ESH = padded edges per core."""
    del N, dt_en, debug
    assert ESH % GATHER_BATCH == 0
    NT = ESH // TILE          # tiles per core
    NSUB = ESH // SUB         # 128-edge subtiles per core
    NCH = ESH // GATHER_BATCH  # DMA chunks per core
    esh_real = E_total // NCORES
    assert E_total % NCORES == 0
    NS = TILE // SUB          # subtiles per tile (4)
    GW = BW + H               # gacc rhs width: [bei | 1 | a2]
    CCOL = GATHER_BATCH // 2  # packed x cols per chunk

    nc = bass.Bass()

    # ---- I/O ----
    xTi = nc.dram_tensor("xTi", [128, ESH // 2], dt_c, kind="ExternalInput")
    beiT = nc.dram_tensor("beiT", [128, NSUB * BW], dt_c, kind="ExternalInput")
    W0d = nc.dram_tensor("W0", [2 * D, H], dt_c, kind="ExternalInput")
    W1d = nc.dram_tensor("W1", [H, H], f32, kind="ExternalInput")
    W2d = nc.dram_tensor("W2", [H, KDIM], f32, kind="ExternalInput")
    b0cd = nc.dram_tensor("b0c", [H, 1], f32, kind="ExternalInput")
    b1rd = nc.dram_tensor("b1r", [1, H], f32, kind="ExternalInput")
    b2rd = nc.dram_tensor("b2r", [1, KDIM], f32, kind="ExternalInput")
    g0cd = nc.dram_tensor("g0c", [H, 1], f32, kind="ExternalInput")
    bt0cd = nc.dram_tensor("bt0c", [H, 1], f32, kind="ExternalInput")
    g1cd = nc.dram_tensor("g1c", [H, 1], f32, kind="ExternalInput")
    bt1cd = nc.dram_tensor("bt1c", [H, 1], f32, kind="ExternalInput")
    rsrd = nc.dram_tensor("rsr", [1, B], f32, kind="ExternalInput")
    invcd = nc.dram_tensor("invc", [B, 1], f32, kind="ExternalInput")
    outd = nc.dram_tensor("out", [B, KDIM], f32, kind="ExternalOutput")

    rg = [list(range(NCORES))]

    with tile.TileContext(nc) as tc:
        with (
            tc.tile_pool(name="const", bufs=1) as cpool,
            tc.tile_pool(name="xp", bufs=3) as xpool,
            tc.tile_pool(name="a1p", bufs=4) as a1pool,
            tc.tile_pool(name="zbp", bufs=3) as zbpool,
            tc.tile_pool(name="a2p", bufs=3) as a2pool,
            tc.tile_pool(name="misc", bufs=2) as mpool,
            tc.tile_pool(name="psA", bufs=2, space="PSUM") as psA,
            tc.tile_pool(name="psB", bufs=2, space="PSUM") as psB,
            tc.tile_pool(name="psG", bufs=1, space="PSUM") as psG,
            tc.tile_pool(name="psS", bufs=2, space="PSUM") as psS,
            tc.tile_pool(name="dram", bufs=1, space="DRAM") as dpool,
        ):
            # ---- constants / params in SBUF ----
            w0sb = cpool.tile([128, H], dt_c)  # W0 duplicated on both halves
            nc.sync.dma_start(w0sb[:], W0d[:])
            w1sb = cpool.tile([H, H], f32)
            nc.sync.dma_start(w1sb[:], W1d[:])
            w2sb = cpool.tile([H, KDIM], f32)
            nc.sync.dma_start(w2sb[:], W2d[:])
            b0c = cpool.tile([H, 1], f32)
            nc.sync.dma_start(b0c[:], b0cd[:])
            b1r = cpool.tile([1, H], f32)
            nc.sync.dma_start(b1r[:], b1rd[:])
            b2r = cpool.tile([1, KDIM], f32)
            nc.sync.dma_start(b2r[:], b2rd[:])
            g0c = cpool.tile([H, 1], f32)
            nc.sync.dma_start(g0c[:], g0cd[:])
            bt0c = cpool.tile([H, 1], f32)
            nc.sync.dma_start(bt0c[:], bt0cd[:])
            g1c = cpool.tile([H, 1], f32)
            nc.sync.dma_start(g1c[:], g1cd[:])
            bt1c = cpool.tile([H, 1], f32)
            nc.sync.dma_start(bt1c[:], bt1cd[:])
            rsr = cpool.tile([1, B], f32)
            nc.sync.dma_start(rsr[:], rsrd[:])
            invc = cpool.tile([B, 1], f32)
            nc.sync.dma_start(invc[:], invcd[:])

            ones_row = cpool.tile([1, H], f32)
            nc.vector.memset(ones_row[:], 1.0)
            id128 = cpool.tile([128, 128], f32)
            make_identity(nc, id128[:])

            # mask for the one subtile that straddles the real/pad boundary
            pad_frac = esh_real % SUB
            edge_mask = None
            if pad_frac:
                pidx = cpool.tile([128, 1], i32)
                nc.gpsimd.iota(pidx[:], pattern=[[0, 1]], base=0,
                               channel_multiplier=1)
                pidx_f = cpool.tile([128, 1], f32)
                nc.vector.tensor_copy(pidx_f[:], pidx[:])
                edge_mask = cpool.tile([128, 1], f32)
                nc.vector.tensor_scalar(
                    edge_mask[:], pidx_f[:], float(pad_frac), None,
                    op0=mybir.AluOpType.is_lt)

            stats1 = cpool.tile([H, 6 * NT], f32)

            # bei resident in SBUF, loaded during pass A
            bei_sb = cpool.tile([128, NSUB * BW], dt_c)
            for ch in range(NCH):
                lo = ch * (GATHER_BATCH // SUB) * BW
                hi = lo + (GATHER_BATCH // SUB) * BW
                nc.sync.dma_start(bei_sb[:, lo:hi], beiT[:, lo:hi])

            # ================= PASS A: BN1 stats =================
            for ch in range(NCH):
                xch = xpool.tile([128, CCOL], dt_c, tag="xch")
                nc.sync.dma_start(xch[:], xTi[:, ch * CCOL:(ch + 1) * CCOL])
                for tp in range(GATHER_BATCH // (2 * TILE)):
                    for u in range(2):
                        t = ch * (GATHER_BATCH // TILE) + 2 * tp + u
                        z1 = psA.tile([H, TILE], f32, space="PSUM", tag="z1")
                        nc.tensor.matmul(
                            z1[:], lhsT=w0sb[u * D:(u + 1) * D, :],
                            rhs=xch[u * D:(u + 1) * D, tp * TILE:(tp + 1) * TILE],
                            start=True, stop=True)
                        a1 = a1pool.tile([H, TILE], dt_c, tag="a1")
                        nc.scalar.activation(
                            a1[:], z1[:], mybir.ActivationFunctionType.Relu,
                            bias=b0c[:, 0:1])
                        pad_lo = esh_real - t * TILE
                        if pad_lo < TILE:
                            nc.vector.memset(a1[:, max(pad_lo, 0):TILE], 0.0)
                        nc.vector.bn_stats(stats1[:, 6 * t:6 * t + 6], a1[:])

            # ---- AllReduce #1: BN1 sums ----
            mv1 = mpool.tile([H, 2], f32, tag="mv")
            nc.vector.bn_aggr(mv1[:], stats1[:])
            # raw sums over this shard (pads are zero -> exact)
            ar1 = mpool.tile([H, 2], f32, tag="ar")
            nc.scalar.mul(ar1[:, 0:1], mv1[:, 0:1], float(ESH))
            msq1 = mpool.tile([H, 1], f32, tag="msq")
            nc.vector.tensor_mul(msq1[:], mv1[:, 0:1], mv1[:, 0:1])
            nc.vector.tensor_add(msq1[:], msq1[:], mv1[:, 1:2])
            nc.scalar.mul(ar1[:, 1:2], msq1[:], float(ESH))

            cc1_in = dpool.tile([H, 2], f32)
            cc1_out = dpool.tile([H, 2], f32)
            nc.sync.dma_start(cc1_in[:], ar1[:])
            nc.gpsimd.collective_compute(
                "AllReduce", mybir.AluOpType.add, replica_groups=rg,
                ins=[cc1_in.opt()], outs=[cc1_out.opt()])
            gs1 = mpool.tile([H, 2], f32, tag="gs")
            nc.sync.dma_start(gs1[:], cc1_out[:])

            # mu, var, s1, t1
            mu1 = mpool.tile([H, 1], f32, tag="mu")
            nc.scalar.mul(mu1[:], gs1[:, 0:1], 1.0 / E_total)
            ex2 = mpool.tile([H, 1], f32, tag="ex2")
            nc.scalar.mul(ex2[:], gs1[:, 1:2], 1.0 / E_total)
            var1 = mpool.tile([H, 1], f32, tag="var")
            nc.vector.tensor_mul(var1[:], mu1[:], mu1[:])
            nc.vector.tensor_sub(var1[:], ex2[:], var1[:])
            sd1 = mpool.tile([H, 1], f32, tag="sd")
            nc.vector.tensor_scalar_add(sd1[:], var1[:], EPS)
            nc.scalar.sqrt(sd1[:], sd1[:])
            isd1 = mpool.tile([H, 1], f32, tag="isd")
            nc.vector.reciprocal(isd1[:], sd1[:])
            s1 = mpool.tile([H, 1], f32, tag="s1")
            nc.vector.tensor_mul(s1[:], g0c[:], isd1[:])
            t1 = mpool.tile([H, 1], f32, tag="t1")
            nc.vector.tensor_mul(t1[:], mu1[:], s1[:])
            nc.vector.tensor_sub(t1[:], bt0c[:], t1[:])

            # W1' (compute dtype), b1' broadcast [128, TILE]
            w1p = cpool.tile([H, H], dt_c)
            nc.vector.tensor_scalar_mul(w1p[:], w1sb[:], s1[:, 0:1])
            pr = psS.tile([1, H], f32, space="PSUM", tag="pss")
            nc.tensor.matmul(pr[:], lhsT=t1[:], rhs=w1sb[:], start=True, stop=True)
            b1p_row = mpool.tile([1, H], f32, tag="b1pr")
            nc.vector.tensor_add(b1p_row[:], pr[:], b1r[:])
            bc_ps = psS.tile([H, H], f32, space="PSUM", tag="pss")
            nc.tensor.matmul(bc_ps[:], lhsT=ones_row[:], rhs=b1p_row[:],
                             start=True, stop=True)
            b1bc = cpool.tile([128, TILE], f32)
            for s in range(NS):
                nc.vector.tensor_copy(b1bc[:, s * H:(s + 1) * H], bc_ps[:])

            # ============ PASS B: a2, G^T / sum2 / Gram accumulation ============
            gacc = psG.tile([H, GW], f32, space="PSUM", tag="gacc")
            for ch in range(NCH):
                xch = xpool.tile([128, CCOL], dt_c, tag="xch")
                nc.sync.dma_start(xch[:], xTi[:, ch * CCOL:(ch + 1) * CCOL])
                for tp in range(GATHER_BATCH // (2 * TILE)):
                    for u in range(2):
                        t = ch * (GATHER_BATCH // TILE) + 2 * tp + u
                        z1 = psA.tile([H, TILE], f32, space="PSUM", tag="z1")
                        nc.tensor.matmul(
                            z1[:], lhsT=w0sb[u * D:(u + 1) * D, :],
                            rhs=xch[u * D:(u + 1) * D, tp * TILE:(tp + 1) * TILE],
                            start=True, stop=True)
                        a1 = a1pool.tile([H, TILE], dt_c, tag="a1")
                        nc.scalar.activation(
                            a1[:], z1[:], mybir.ActivationFunctionType.Relu,
                            bias=b0c[:, 0:1])
                        z2 = psB.tile([H, TILE], f32, space="PSUM", tag="z2")
                        for s in range(NS):
                            nc.tensor.matmul(
                                z2[:, s * H:(s + 1) * H],
                                lhsT=a1[:, s * SUB:(s + 1) * SUB],
                                rhs=w1p[:], start=True, stop=True)
                        z2b = zbpool.tile([128, TILE], dt_c, tag="z2b")
                        nc.vector.tensor_add(z2b[:], z2[:], b1bc[:])
                        # a2t: per subtile [bei (32) | ones (1) | a2 (128)]
                        a2t = a2pool.tile([128, NS * GW], dt_c, tag="a2t")
                        a2t3 = a2t[:].rearrange("p (g c) -> p g c", c=GW)
                        nc.gpsimd.tensor_copy(
                            a2t3[:, :, 0:BW],
                            bei_sb[:, (t * NS) * BW:(t * NS + NS) * BW]
                            .rearrange("p (g c) -> p g c", c=BW))
                        nc.gpsimd.tensor_scalar_max(
                            a2t3[:, :, BW:GW],
                            z2b[:].rearrange("p (g c) -> p g c", c=H), 0.0)
                        # zero a2 for pad edges (bei cols are host-zeroed)
                        for s in range(NS):
                            pl = esh_real - (t * NS + s) * SUB
                            if pl <= 0:
                                nc.vector.memset(a2t3[:, s, BW:GW], 0.0)
                            elif pl < SUB:
                                nc.vector.tensor_scalar_mul(
                                    a2t3[:, s, BW:GW], a2t3[:, s, BW:GW],
                                    edge_mask[:, 0:1])
                        first = (t == 0)
                        last = (t == NT - 1)
                        for s in range(NS):
                            nc.tensor.matmul(
                                gacc[:],
                                lhsT=a2t[:, s * GW + BW:(s + 1) * GW],
                                rhs=a2t[:, s * GW:(s + 1) * GW],
                                start=(first and s == 0),
                                stop=(last and s == NS - 1),
                                skip_group_check=True)

            # ---- AllReduce #2: [G^T | sum2 | sumsq2] ----
            garr = mpool.tile([H, BW + 1], f32, tag="garr")
            nc.vector.tensor_copy(garr[:, 0:BW], gacc[:, 0:BW])
            scr = mpool.tile([128, 128], f32, tag="scr")
            nc.vector.tensor_mul(scr[:], gacc[:, BW:GW], id128[:])
            nc.vector.tensor_reduce(
                garr[:, BW:BW + 1], scr[:], mybir.AxisListType.X,
                mybir.AluOpType.add)

            cc2_in = dpool.tile([H, BW + 1], f32)
            cc2_out = dpool.tile([H, BW + 1], f32)
            nc.sync.dma_start(cc2_in[:], garr[:])
            nc.gpsimd.collective_compute(
                "AllReduce", mybir.AluOpType.add, replica_groups=rg,
                ins=[cc2_in.opt()], outs=[cc2_out.opt()])
            gall = mpool.tile([H, BW + 1], f32, tag="gall")
            nc.sync.dma_start(gall[:], cc2_out[:])

            # ---- epilogue ----
            mu2 = mpool.tile([H, 1], f32, tag="mu")
            nc.scalar.mul(mu2[:], gall[:, B:B + 1], 1.0 / E_total)
            ex2b = mpool.tile([H, 1], f32, tag="ex2")
            nc.scalar.mul(ex2b[:], gall[:, BW:BW + 1], 1.0 / E_total)
            var2 = mpool.tile([H, 1], f32, tag="var")
            nc.vector.tensor_mul(var2[:], mu2[:], mu2[:])
            nc.vector.tensor_sub(var2[:], ex2b[:], var2[:])
            sd2 = mpool.tile([H, 1], f32, tag="sd")
            nc.vector.tensor_scalar_add(sd2[:], var2[:], EPS)
            nc.scalar.sqrt(sd2[:], sd2[:])
            isd2 = mpool.tile([H, 1], f32, tag="isd")
            nc.vector.reciprocal(isd2[:], sd2[:])
            s2 = mpool.tile([H, 1], f32, tag="s1")
            nc.vector.tensor_mul(s2[:], g1c[:], isd2[:])
            t2 = mpool.tile([H, 1], f32, tag="t1")
            nc.vector.tensor_mul(t2[:], mu2[:], s2[:])
            nc.vector.tensor_sub(t2[:], bt1c[:], t2[:])

            w2p = mpool.tile([H, KDIM], f32, tag="w2p")
            nc.vector.tensor_scalar_mul(w2p[:], w2sb[:], s2[:, 0:1])
            pr2 = psS.tile([1, KDIM], f32, space="PSUM", tag="pss")
            nc.tensor.matmul(pr2[:], lhsT=t2[:], rhs=w2sb[:], start=True, stop=True)
            b2p_row = mpool.tile([1, KDIM], f32, tag="b2pr")
            nc.vector.tensor_add(b2p_row[:], pr2[:], b2r[:])

            out_ps = psS.tile([B, KDIM], f32, space="PSUM", tag="pss")
            nc.tensor.matmul(out_ps[:], lhsT=gall[:, 0:B], rhs=w2p[:],
                             start=True, stop=False)
            nc.tensor.matmul(out_ps[:], lhsT=rsr[:], rhs=b2p_row[:],
                             start=False, stop=True)
            outsb = mpool.tile([B, KDIM], f32, tag="outsb")
            nc.vector.tensor_scalar_mul(outsb[:], out_ps[:], invc[:, 0:1])
            nc.sync.dma_start(outd[:], outsb[:])

    # Legalize waits for walrus (TRN2: max 1 wait/instruction; extras are
    # spilled onto ldweights / event-semaphore instructions).
    import bass_rust as _br
    _br.move_matmul_waits_to_ldweights(nc.m)
    _br.generate_event_semaphores(nc)
    nc.finalize()
    return nc


def _ceil_to(x, m):
    return (x + m - 1) // m * m


def make_inputs(inputs, ESH, N, dt_c=bf16, dt_en=None):
    """Host-side shard/layout prep. Returns in_maps for run_bass_kernel_spmd."""
    del N, dt_en
    np_c = _np_dt(dt_c)
    en = np.asarray(inputs["edge_nodes"], dtype=np.float32)
    x = np.asarray(inputs["edge_feats"], dtype=np.float32)
    src = np.asarray(inputs["src"]).astype(np.int64)
    dst = np.asarray(inputs["dst"]).astype(np.int64)
    E = x.shape[0]
    Nn = en.shape[1]
    esh_real = E // NCORES
    NSUB = ESH // SUB

    # exact row sums via degree counts (en entries are 0/1)
    deg = (np.bincount(src, minlength=Nn) + np.bincount(dst, minlength=Nn))
    rs = en.astype(np.float64) @ deg.astype(np.float64)
    inv = np.where(rs > 0, 1.0 / np.where(rs > 0, rs, 1.0), 0.0)

    enT = en.T  # [N, B]

    common = dict(
        W0=np.vstack([np.asarray(inputs["W0"], np.float32)] * 2).astype(np_c),
        W1=np.asarray(inputs["W1"], np.float32),
        W2=np.asarray(inputs["W2"], np.float32),
        b0c=np.asarray(inputs["b0"], np.float32).reshape(H, 1),
        b1r=np.asarray(inputs["b1"], np.float32).reshape(1, H),
        b2r=np.asarray(inputs["b2"], np.float32).reshape(1, KDIM),
        g0c=np.asarray(inputs["g0"], np.float32).reshape(H, 1),
        bt0c=np.asarray(inputs["bt0"], np.float32).reshape(H, 1),
        g1c=np.asarray(inputs["g1"], np.float32).reshape(H, 1),
        bt1c=np.asarray(inputs["bt1"], np.float32).reshape(H, 1),
        rsr=rs.astype(np.float32).reshape(1, B),
        invc=inv.astype(np.float32).reshape(B, 1),
    )

    in_maps = []
    for c in range(NCORES):
        lo = c * esh_real
        xs = x[lo:lo + esh_real]
        xT = np.zeros((D, ESH), np.float32)
        xT[:, :esh_real] = xs.T
        NTP = ESH // (2 * TILE)
        xTi = np.ascontiguousarray(
            xT.reshape(D, NTP, 2, TILE).transpose(2, 0, 1, 3).reshape(128, ESH // 2)
        ).astype(np_c)

        bei_aug = np.zeros((ESH, BW), np.float32)
        bei_aug[:esh_real, 0:B] = enT[src[lo:lo + esh_real]] + enT[dst[lo:lo + esh_real]]
        bei_aug[:esh_real, B] = 1.0
        beiT = np.ascontiguousarray(
            bei_aug.reshape(NSUB, 128, BW).transpose(1, 0, 2).reshape(128, NSUB * BW)
        ).astype(np_c)

        in_maps.append(dict(common, xTi=xTi, beiT=beiT))
    return in_maps


_NC_CACHE = {}


def kernel(**inputs):
    dt_c = bf16 if COMPUTE_DT == "bf16" else f32
    x = np.asarray(inputs["edge_feats"])
    en = np.asarray(inputs["edge_nodes"])
    E = x.shape[0]
    N = en.shape[1]
    ESH = _ceil_to(E // NCORES, GATHER_BATCH)
    key = (ESH, N, E, COMPUTE_DT)
    if key not in _NC_CACHE:
        _NC_CACHE[key] = build_nc(ESH, N, E, dt_c=dt_c)
    nc = _NC_CACHE[key]
    in_maps = make_inputs(inputs, ESH, N, dt_c=dt_c)
    res = run_bass_kernel_spmd(nc, in_maps, list(range(NCORES)))
    return np.asarray(res.results[0]["out"], np.float32)


# revision 7
# speedup vs baseline: 2.3801x; 2.2268x over previous
"""Trainium2 Bass kernel for DeepEdgeConvolution (gnn_message_passing).

Math (reference):
    bei = edge_nodes[:, src] + edge_nodes[:, dst]          # [B, E]
    bei = bei / row_sum (0 if empty row)
    h = BN1(relu(x @ W0 + b0)); h = BN2(relu(h @ W1 + b1)); h = h @ W2 + b2
    out = bei @ h                                          # [B, K]

Restructured: fold BN1 into (W1, b1) and BN2 into (W2, b2):
    a1 = relu(x @ W0 + b0)             (BN1 stats over E -> s1, t1)
    W1' = diag(s1) W1 ; b1' = t1 @ W1 + b1
    a2 = relu(a1 @ W1' + b1')          (BN2 stats over E -> s2, t2)
    W2' = diag(s2) W2 ; b2' = t2 @ W2 + b2
    out = diag(inv) [ (bei_raw @ a2) @ W2' + rs_raw x b2' ]

Sharding: edges across 8 cores; two streaming passes over x^T per core
(pass A: BN1 stats via bn_stats; pass B: recompute a1, then a2, G).

bei is computed on the HOST (sharding the columns of batch_edge_idcs per the
sharding hint): beiT is streamed as a dense packed input [128, NSUB*33] where
each 33-col block is [bei | 1] for one 128-edge subtile (edges on partitions).
Pad edges get all-zero columns (including the ones entry), which kills every
pad correction. Row sums rs / inv are exact small host-side inputs.

G accumulation (the [B,E]x[E,K] spmm): per 128-edge subtile,
    psG += a2_sub^T @ [bei_aug | a2_sub]    -> [H, 33 + H]
giving G^T (cols 0:32), sum(a2) (col 32, via the ones column) and the a2 Gram
matrix whose diagonal is sum(a2^2) -- one PSUM chain yields everything BN2 and
the final matmul need.  One AllReduce of [H, 34] follows; the epilogue is a
couple of tiny matmuls.
"""

import numpy as np

import concourse.bacc as bacc
import concourse.bass as bass
import concourse.tile as tile
from concourse import mybir
from concourse.bass_utils import run_bass_kernel_spmd
from concourse.masks import make_identity

f32 = mybir.dt.float32
bf16 = mybir.dt.bfloat16
i32 = mybir.dt.int32

NCORES = 8
B, D, H, KDIM = 32, 64, 128, 128
EPS = 1e-5
TILE = 512           # edges per tile
SUB = 128            # edges per matmul subtile
GATHER_BATCH = 2048  # edges per DMA chunk (ESH must be a multiple)

# compute dtype: "f32" (exact-ish) or "bf16" (fast).
COMPUTE_DT = "bf16"

BW = B + 1           # bei block width: [bei (32) | ones (1)]


def _np_dt(dt):
    if dt == bf16:
        import ml_dtypes
        return ml_dtypes.bfloat16
    return np.float32


def build_nc(ESH, N, E_total, dt_c=bf16, dt_en=None, debug=False):
    """Build the SPMD Bass program. ESH = padded edges per core."""
    del N, dt_en, debug
    assert ESH % GATHER_BATCH == 0
    NT = ESH // TILE          # tiles per core
    NSUB = ESH // SUB         # 128-edge subtiles per core
    NCH = ESH // GATHER_BATCH  # DMA chunks per core
    esh_real = E_total // NCORES
    assert E_total % NCORES == 0
    NS = TILE // SUB          # subtiles per tile (4)
    GW = BW + H               # gacc rhs width: [bei | 1 | a2]
    CCOL = GATHER_BATCH // 2  # packed x cols per chunk

    nc = bass.Bass()

    # ---- I/O ----
    xTi = nc.dram_tensor("xTi", [128, ESH // 2], dt_c, kind="ExternalInput")
    beiT = nc.dram_tensor("beiT", [128, NSUB * BW], dt_c, kind="ExternalInput")
    W0d = nc.dram_tensor("W0", [2 * D, H], dt_c, kind="ExternalInput")
    W1d = nc.dram_tensor("W1", [H, H], f32, kind="ExternalInput")
    W2d = nc.dram_tensor("W2", [H, KDIM], f32, kind="ExternalInput")
    b0cd = nc.dram_tensor("b0c", [H, 1], f32, kind="ExternalInput")
    b1rd = nc.dram_tensor("b1r", [1, H], f32, kind="ExternalInput")
    b2rd = nc.dram_tensor("b2r", [1, KDIM], f32, kind="ExternalInput")
    g0cd = nc.dram_tensor("g0c", [H, 1], f32, kind="ExternalInput")
    bt0cd = nc.dram_tensor("bt0c", [H, 1], f32, kind="ExternalInput")
    g1cd = nc.dram_tensor("g1c", [H, 1], f32, kind="ExternalInput")
    bt1cd = nc.dram_tensor("bt1c", [H, 1], f32, kind="ExternalInput")
    rsrd = nc.dram_tensor("rsr", [1, B], f32, kind="ExternalInput")
    invcd = nc.dram_tensor("invc", [B, 1], f32, kind="ExternalInput")
    outd = nc.dram_tensor("out", [B, KDIM], f32, kind="ExternalOutput")

    rg = [list(range(NCORES))]

    with tile.TileContext(nc) as tc:
        with (
            tc.tile_pool(name="const", bufs=1) as cpool,
            tc.tile_pool(name="xp", bufs=3) as xpool,
            tc.tile_pool(name="a1p", bufs=4) as a1pool,
            tc.tile_pool(name="zbp", bufs=3) as zbpool,
            tc.tile_pool(name="a2p", bufs=3) as a2pool,
            tc.tile_pool(name="misc", bufs=2) as mpool,
            tc.tile_pool(name="psA", bufs=2, space="PSUM") as psA,
            tc.tile_pool(name="psB", bufs=2, space="PSUM") as psB,
            tc.tile_pool(name="psG", bufs=1, space="PSUM") as psG,
            tc.tile_pool(name="psS", bufs=2, space="PSUM") as psS,
            tc.tile_pool(name="dram", bufs=1, space="DRAM") as dpool,
        ):
            # ---- constants / params in SBUF ----
            w0sb = cpool.tile([128, H], dt_c)  # W0 duplicated on both halves
            nc.sync.dma_start(w0sb[:], W0d[:])
            w1sb = cpool.tile([H, H], f32)
            nc.sync.dma_start(w1sb[:], W1d[:])
            w2sb = cpool.tile([H, KDIM], f32)
            nc.sync.dma_start(w2sb[:], W2d[:])
            b0c = cpool.tile([H, 1], f32)
            nc.sync.dma_start(b0c[:], b0cd[:])
            b1r = cpool.tile([1, H], f32)
            nc.sync.dma_start(b1r[:], b1rd[:])
            b2r = cpool.tile([1, KDIM], f32)
            nc.sync.dma_start(b2r[:], b2rd[:])
            g0c = cpool.tile([H, 1], f32)
            nc.sync.dma_start(g0c[:], g0cd[:])
            bt0c = cpool.tile([H, 1], f32)
            nc.sync.dma_start(bt0c[:], bt0cd[:])
            g1c = cpool.tile([H, 1], f32)
            nc.sync.dma_start(g1c[:], g1cd[:])
            bt1c = cpool.tile([H, 1], f32)
            nc.sync.dma_start(bt1c[:], bt1cd[:])
            rsr = cpool.tile([1, B], f32)
            nc.sync.dma_start(rsr[:], rsrd[:])
            invc = cpool.tile([B, 1], f32)
            nc.sync.dma_start(invc[:], invcd[:])

            ones_row = cpool.tile([1, H], f32)
            nc.vector.memset(ones_row[:], 1.0)
            id128 = cpool.tile([128, 128], f32)
            make_identity(nc, id128[:])

            # mask for the one subtile that straddles the real/pad boundary
            pad_frac = esh_real % SUB
            edge_mask = None
            if pad_frac:
                pidx = cpool.tile([128, 1], i32)
                nc.gpsimd.iota(pidx[:], pattern=[[0, 1]], base=0,
                               channel_multiplier=1)
                pidx_f = cpool.tile([128, 1], f32)
                nc.vector.tensor_copy(pidx_f[:], pidx[:])
                edge_mask = cpool.tile([128, 1], f32)
                nc.vector.tensor_scalar(
                    edge_mask[:], pidx_f[:], float(pad_frac), None,
                    op0=mybir.AluOpType.is_lt)

            stats1 = cpool.tile([H, 6 * NT], f32)

            # bei resident in SBUF, loaded during pass A
            bei_sb = cpool.tile([128, NSUB * BW], dt_c)
            for ch in range(NCH):
                lo = ch * (GATHER_BATCH // SUB) * BW
                hi = lo + (GATHER_BATCH // SUB) * BW
                nc.sync.dma_start(bei_sb[:, lo:hi], beiT[:, lo:hi])

            # ================= PASS A: BN1 stats =================
            for ch in range(NCH):
                xch = xpool.tile([128, CCOL], dt_c, tag="xch")
                nc.sync.dma_start(xch[:], xTi[:, ch * CCOL:(ch + 1) * CCOL])
                for tp in range(GATHER_BATCH // (2 * TILE)):
                    for u in range(2):
                        t = ch * (GATHER_BATCH // TILE) + 2 * tp + u
                        z1 = psA.tile([H, TILE], f32, space="PSUM", tag="z1")
                        nc.tensor.matmul(
                            z1[:], lhsT=w0sb[u * D:(u + 1) * D, :],
                            rhs=xch[u * D:(u + 1) * D, tp * TILE:(tp + 1) * TILE],
                            start=True, stop=True)
                        a1 = a1pool.tile([H, TILE], dt_c, tag="a1")
                        nc.scalar.activation(
                            a1[:], z1[:], mybir.ActivationFunctionType.Relu,
                            bias=b0c[:, 0:1])
                        pad_lo = esh_real - t * TILE
                        if pad_lo < TILE:
                            nc.vector.memset(a1[:, max(pad_lo, 0):TILE], 0.0)
                        nc.vector.bn_stats(stats1[:, 6 * t:6 * t + 6], a1[:])

            # ---- AllReduce #1: BN1 sums ----
            mv1 = mpool.tile([H, 2], f32, tag="mv")
            nc.vector.bn_aggr(mv1[:], stats1[:])
            # raw sums over this shard (pads are zero -> exact)
            ar1 = mpool.tile([H, 2], f32, tag="ar")
            nc.scalar.mul(ar1[:, 0:1], mv1[:, 0:1], float(ESH))
            msq1 = mpool.tile([H, 1], f32, tag="msq")
            nc.vector.tensor_mul(msq1[:], mv1[:, 0:1], mv1[:, 0:1])
            nc.vector.tensor_add(msq1[:], msq1[:], mv1[:, 1:2])
            nc.scalar.mul(ar1[:, 1:2], msq1[:], float(ESH))

            cc1_in = dpool.tile([H, 2], f32)
            cc1_out = dpool.tile([H, 2], f32)
            nc.sync.dma_start(cc1_in[:], ar1[:])
            nc.gpsimd.collective_compute(
                "AllReduce", mybir.AluOpType.add, replica_groups=rg,
                ins=[cc1_in.opt()], outs=[cc1_out.opt()])
            gs1 = mpool.tile([H, 2], f32, tag="gs")
            nc.sync.dma_start(gs1[:], cc1_out[:])

            # mu, var, s1, t1
            mu1 = mpool.tile([H, 1], f32, tag="mu")
            nc.scalar.mul(mu1[:], gs1[:, 0:1], 1.0 / E_total)
            ex2 = mpool.tile([H, 1], f32, tag="ex2")
            nc.scalar.mul(ex2[:], gs1[:, 1:2], 1.0 / E_total)
            var1 = mpool.tile([H, 1], f32, tag="var")
            nc.vector.tensor_mul(var1[:], mu1[:], mu1[:])
            nc.vector.tensor_sub(var1[:], ex2[:], var1[:])
            sd1 = mpool.tile([H, 1], f32, tag="sd")
            nc.vector.tensor_scalar_add(sd1[:], var1[:], EPS)
            nc.scalar.sqrt(sd1[:], sd1[:])
            isd1 = mpool.tile([H, 1], f32, tag="isd")
            nc.vector.reciprocal(isd1[:], sd1[:])
            s1 = mpool.tile([H, 1], f32, tag="s1")
            nc.vector.tensor_mul(s1[:], g0c[:], isd1[:])
            t1 = mpool.tile([H, 1], f32, tag="t1")
            nc.vector.tensor_mul(t1[:], mu1[:], s1[:])
            nc.vector.tensor_sub(t1[:], bt0c[:], t1[:])

            # W1' (compute dtype), b1' tiled [1, TILE] (bias enters z2 PSUM
            # via a K=1 matmul)
            w1p = cpool.tile([H, H], dt_c)
            nc.vector.tensor_scalar_mul(w1p[:], w1sb[:], s1[:, 0:1])
            pr = psS.tile([1, H], f32, space="PSUM", tag="pss")
            nc.tensor.matmul(pr[:], lhsT=t1[:], rhs=w1sb[:], start=True, stop=True)
            b1tile = cpool.tile([1, TILE], f32)
            for s in range(NS):
                nc.vector.tensor_add(b1tile[:, s * H:(s + 1) * H], pr[:], b1r[:])

            # ============ PASS B: a2, G^T / sum2 / Gram accumulation ============
            gacc = psG.tile([H, GW], f32, space="PSUM", tag="gacc")
            for ch in range(NCH):
                xch = xpool.tile([128, CCOL], dt_c, tag="xch")
                nc.sync.dma_start(xch[:], xTi[:, ch * CCOL:(ch + 1) * CCOL])
                for tp in range(GATHER_BATCH // (2 * TILE)):
                    for u in range(2):
                        t = ch * (GATHER_BATCH // TILE) + 2 * tp + u
                        z1 = psA.tile([H, TILE], f32, space="PSUM", tag="z1")
                        nc.tensor.matmul(
                            z1[:], lhsT=w0sb[u * D:(u + 1) * D, :],
                            rhs=xch[u * D:(u + 1) * D, tp * TILE:(tp + 1) * TILE],
                            start=True, stop=True)
                        a1 = a1pool.tile([H, TILE], dt_c, tag="a1")
                        nc.vector.tensor_scalar(
                            a1[:], z1[:], b0c[:, 0:1], 0.0,
                            op0=mybir.AluOpType.add, op1=mybir.AluOpType.max)
                        z2 = psB.tile([H, TILE], f32, space="PSUM", tag="z2")
                        nc.tensor.matmul(
                            z2[:], lhsT=ones_row[:], rhs=b1tile[:],
                            start=True, stop=False, skip_group_check=True)
                        for s in range(NS):
                            nc.tensor.matmul(
                                z2[:, s * H:(s + 1) * H],
                                lhsT=a1[:, s * SUB:(s + 1) * SUB],
                                rhs=w1p[:], start=False, stop=(s == NS - 1),
                                skip_group_check=True)
                        # a2t: per subtile [bei (32) | ones (1) | a2 (128)]
                        a2t = a2pool.tile([128, NS * GW], dt_c, tag="a2t")
                        a2t3 = a2t[:].rearrange("p (g c) -> p g c", c=GW)
                        nc.vector.tensor_copy(
                            a2t3[:, :, 0:BW],
                            bei_sb[:, (t * NS) * BW:(t * NS + NS) * BW]
                            .rearrange("p (g c) -> p g c", c=BW))
                        nc.scalar.activation(
                            a2t3[:, :, BW:GW],
                            z2[:].rearrange("p (g c) -> p g c", c=H),
                            mybir.ActivationFunctionType.Relu)
                        # zero a2 for pad edges (bei cols are host-zeroed)
                        for s in range(NS):
                            pl = esh_real - (t * NS + s) * SUB
                            if pl <= 0:
                                nc.vector.memset(a2t3[:, s, BW:GW], 0.0)
                            elif pl < SUB:
                                nc.vector.tensor_scalar_mul(
                                    a2t3[:, s, BW:GW], a2t3[:, s, BW:GW],
                                    edge_mask[:, 0:1])
                        first = (t == 0)
                        last = (t == NT - 1)
                        for s in range(NS):
                            nc.tensor.matmul(
                                gacc[:],
                                lhsT=a2t[:, s * GW + BW:(s + 1) * GW],
                                rhs=a2t[:, s * GW:(s + 1) * GW],
                                start=(first and s == 0),
                                stop=(last and s == NS - 1),
                                skip_group_check=True)

            # ---- AllReduce #2: [G^T | sum2 | sumsq2] ----
            garr = mpool.tile([H, BW + 1], f32, tag="garr")
            nc.vector.tensor_copy(garr[:, 0:BW], gacc[:, 0:BW])
            scr = mpool.tile([128, 128], f32, tag="scr")
            nc.vector.tensor_mul(scr[:], gacc[:, BW:GW], id128[:])
            nc.vector.tensor_reduce(
                garr[:, BW:BW + 1], scr[:], mybir.AxisListType.X,
                mybir.AluOpType.add)

            cc2_in = dpool.tile([H, BW + 1], f32)
            cc2_out = dpool.tile([H, BW + 1], f32)
            nc.sync.dma_start(cc2_in[:], garr[:])
            nc.gpsimd.collective_compute(
                "AllReduce", mybir.AluOpType.add, replica_groups=rg,
                ins=[cc2_in.opt()], outs=[cc2_out.opt()])
            gall = mpool.tile([H, BW + 1], f32, tag="gall")
            nc.sync.dma_start(gall[:], cc2_out[:])

            # ---- epilogue ----
            mu2 = mpool.tile([H, 1], f32, tag="mu")
            nc.scalar.mul(mu2[:], gall[:, B:B + 1], 1.0 / E_total)
            ex2b = mpool.tile([H, 1], f32, tag="ex2")
            nc.scalar.mul(ex2b[:], gall[:, BW:BW + 1], 1.0 / E_total)
            var2 = mpool.tile([H, 1], f32, tag="var")
            nc.vector.tensor_mul(var2[:], mu2[:], mu2[:])
            nc.vector.tensor_sub(var2[:], ex2b[:], var2[:])
            sd2 = mpool.tile([H, 1], f32, tag="sd")
            nc.vector.tensor_scalar_add(sd2[:], var2[:], EPS)
            nc.scalar.sqrt(sd2[:], sd2[:])
            isd2 = mpool.tile([H, 1], f32, tag="isd")
            nc.vector.reciprocal(isd2[:], sd2[:])
            s2 = mpool.tile([H, 1], f32, tag="s1")
            nc.vector.tensor_mul(s2[:], g1c[:], isd2[:])
            t2 = mpool.tile([H, 1], f32, tag="t1")
            nc.vector.tensor_mul(t2[:], mu2[:], s2[:])
            nc.vector.tensor_sub(t2[:], bt1c[:], t2[:])

            w2p = mpool.tile([H, KDIM], f32, tag="w2p")
            nc.vector.tensor_scalar_mul(w2p[:], w2sb[:], s2[:, 0:1])
            pr2 = psS.tile([1, KDIM], f32, space="PSUM", tag="pss")
            nc.tensor.matmul(pr2[:], lhsT=t2[:], rhs=w2sb[:], start=True, stop=True)
            b2p_row = mpool.tile([1, KDIM], f32, tag="b2pr")
            nc.vector.tensor_add(b2p_row[:], pr2[:], b2r[:])

            out_ps = psS.tile([B, KDIM], f32, space="PSUM", tag="pss")
            nc.tensor.matmul(out_ps[:], lhsT=gall[:, 0:B], rhs=w2p[:],
                             start=True, stop=False)
            nc.tensor.matmul(out_ps[:], lhsT=rsr[:], rhs=b2p_row[:],
                             start=False, stop=True)
            outsb = mpool.tile([B, KDIM], f32, tag="outsb")
            nc.vector.tensor_scalar_mul(outsb[:], out_ps[:], invc[:, 0:1])
            nc.sync.dma_start(outd[:], outsb[:])

    # Legalize waits for walrus (TRN2: max 1 wait/instruction; extras are
    # spilled onto ldweights / event-semaphore instructions).
    import bass_rust as _br
    _br.move_matmul_waits_to_ldweights(nc.m)
    _br.generate_event_semaphores(nc)
    nc.finalize()
    return nc


def _ceil_to(x, m):
    return (x + m - 1) // m * m


def make_inputs(inputs, ESH, N, dt_c=bf16, dt_en=None):
    """Host-side shard/layout prep. Returns in_maps for run_bass_kernel_spmd."""
    del N, dt_en
    np_c = _np_dt(dt_c)
    en = np.asarray(inputs["edge_nodes"], dtype=np.float32)
    x = np.asarray(inputs["edge_feats"], dtype=np.float32)
    src = np.asarray(inputs["src"]).astype(np.int64)
    dst = np.asarray(inputs["dst"]).astype(np.int64)
    E = x.shape[0]
    Nn = en.shape[1]
    esh_real = E // NCORES
    NSUB = ESH // SUB

    # exact row sums via degree counts (en entries are 0/1)
    deg = (np.bincount(src, minlength=Nn) + np.bincount(dst, minlength=Nn))
    rs = en.astype(np.float64) @ deg.astype(np.float64)
    inv = np.where(rs > 0, 1.0 / np.where(rs > 0, rs, 1.0), 0.0)

    enT = en.T  # [N, B]

    common = dict(
        W0=np.vstack([np.asarray(inputs["W0"], np.float32)] * 2).astype(np_c),
        W1=np.asarray(inputs["W1"], np.float32),
        W2=np.asarray(inputs["W2"], np.float32),
        b0c=np.asarray(inputs["b0"], np.float32).reshape(H, 1),
        b1r=np.asarray(inputs["b1"], np.float32).reshape(1, H),
        b2r=np.asarray(inputs["b2"], np.float32).reshape(1, KDIM),
        g0c=np.asarray(inputs["g0"], np.float32).reshape(H, 1),
        bt0c=np.asarray(inputs["bt0"], np.float32).reshape(H, 1),
        g1c=np.asarray(inputs["g1"], np.float32).reshape(H, 1),
        bt1c=np.asarray(inputs["bt1"], np.float32).reshape(H, 1),
        rsr=rs.astype(np.float32).reshape(1, B),
        invc=inv.astype(np.float32).reshape(B, 1),
    )

    in_maps = []
    for c in range(NCORES):
        lo = c * esh_real
        xs = x[lo:lo + esh_real]
        xT = np.zeros((D, ESH), np.float32)
        xT[:, :esh_real] = xs.T
        NTP = ESH // (2 * TILE)
        xTi = np.ascontiguousarray(
            xT.reshape(D, NTP, 2, TILE).transpose(2, 0, 1, 3).reshape(128, ESH // 2)
        ).astype(np_c)

        bei_aug = np.zeros((ESH, BW), np.float32)
        bei_aug[:esh_real, 0:B] = enT[src[lo:lo + esh_real]] + enT[dst[lo:lo + esh_real]]
        bei_aug[:esh_real, B] = 1.0
        beiT = np.ascontiguousarray(
            bei_aug.reshape(NSUB, 128, BW).transpose(1, 0, 2).reshape(128, NSUB * BW)
        ).astype(np_c)

        in_maps.append(dict(common, xTi=xTi, beiT=beiT))
    return in_maps


_NC_CACHE = {}


def kernel(**inputs):
    dt_c = bf16 if COMPUTE_DT == "bf16" else f32
    x = np.asarray(inputs["edge_feats"])
    en = np.asarray(inputs["edge_nodes"])
    E = x.shape[0]
    N = en.shape[1]
    ESH = _ceil_to(E // NCORES, GATHER_BATCH)
    key = (ESH, N, E, COMPUTE_DT)
    if key not in _NC_CACHE:
        _NC_CACHE[key] = build_nc(ESH, N, E, dt_c=dt_c)
    nc = _NC_CACHE[key]
    in_maps = make_inputs(inputs, ESH, N, dt_c=dt_c)
    res = run_bass_kernel_spmd(nc, in_maps, list(range(NCORES)))
    return np.asarray(res.results[0]["out"], np.float32)


# revision 13
# speedup vs baseline: 3.6716x; 1.5426x over previous
"""Trainium2 Bass kernel for DeepEdgeConvolution (gnn_message_passing).

Math (reference):
    bei = edge_nodes[:, src] + edge_nodes[:, dst]          # [B, E]
    bei = bei / row_sum (0 if empty row)
    h = BN1(relu(x @ W0 + b0)); h = BN2(relu(h @ W1 + b1)); h = h @ W2 + b2
    out = bei @ h                                          # [B, K]

Restructured: fold BN1 into (W1, b1) and BN2 into (W2, b2):
    a1 = relu(x @ W0 + b0)             (BN1 stats over E -> s1, t1)
    W1' = diag(s1) W1 ; b1' = t1 @ W1 + b1
    a2 = relu(a1 @ W1' + b1')          (BN2 stats over E -> s2, t2)
    W2' = diag(s2) W2 ; b2' = t2 @ W2 + b2
    out = diag(inv) [ (bei_raw @ a2) @ W2' + rs_raw x b2' ]

Sharding: edges across 8 cores; two streaming passes over x^T per core
(pass A: BN1 stats via bn_stats; pass B: recompute a1, then a2, G).

bei is computed on the HOST (sharding the columns of batch_edge_idcs per the
sharding hint): beiT is streamed as a dense packed input [128, NSUB*33] where
each 33-col block is [bei | 1] for one 128-edge subtile (edges on partitions).
Pad edges get all-zero columns (including the ones entry), which kills every
pad correction. Row sums rs / inv are exact small host-side inputs.

G accumulation (the [B,E]x[E,K] spmm): per 128-edge subtile,
    psG += a2_sub^T @ [bei_aug | a2_sub]    -> [H, 33 + H]
giving G^T (cols 0:32), sum(a2) (col 32, via the ones column) and the a2 Gram
matrix whose diagonal is sum(a2^2) -- one PSUM chain yields everything BN2 and
the final matmul need.  One AllReduce of [H, 34] follows; the epilogue is a
couple of tiny matmuls.
"""

import numpy as np

import concourse.bacc as bacc
import concourse.bass as bass
import concourse.tile as tile
from concourse import mybir
from concourse.bass_utils import run_bass_kernel_spmd
from concourse.masks import make_identity

f32 = mybir.dt.float32
bf16 = mybir.dt.bfloat16
i32 = mybir.dt.int32

NCORES = 8
B, D, H, KDIM = 32, 64, 128, 128
EPS = 1e-5
TILE = 512           # edges per tile
SUB = 128            # edges per matmul subtile
GATHER_BATCH = 2048  # edges per DMA chunk (ESH must be a multiple)

# compute dtype: "f32" (exact-ish) or "bf16" (fast).
COMPUTE_DT = "bf16"

BW = B + 1           # bei block width: [bei (32) | ones (1)]


def _np_dt(dt):
    if dt == bf16:
        import ml_dtypes
        return ml_dtypes.bfloat16
    return np.float32


def build_nc(ESH, N, E_total, dt_c=bf16, dt_en=None, debug=False):
    """Build the SPMD Bass program. ESH = padded edges per core."""
    del N, dt_en, debug
    assert ESH % GATHER_BATCH == 0
    NT = ESH // TILE          # tiles per core
    NSUB = ESH // SUB         # 128-edge subtiles per core
    NCH = ESH // GATHER_BATCH  # DMA chunks per core
    esh_real = E_total // NCORES
    assert E_total % NCORES == 0
    NS = TILE // SUB          # subtiles per tile (4)
    GW = BW + H               # gacc rhs width: [bei | 1 | a2]
    CCOL = GATHER_BATCH // 2  # packed x cols per chunk
    PRE = min(48, NT - NT % 4)  # pass-B prelude tiles (hide AllReduce #1)

    nc = bass.Bass()

    # ---- I/O ----
    xTi = nc.dram_tensor("xTi", [128, ESH // 2], dt_c, kind="ExternalInput")
    beiT = nc.dram_tensor("beiT", [128, NSUB * BW], dt_c, kind="ExternalInput")
    W0d = nc.dram_tensor("W0", [2 * D, H], dt_c, kind="ExternalInput")
    W1d = nc.dram_tensor("W1", [H, H], f32, kind="ExternalInput")
    W2d = nc.dram_tensor("W2", [H, KDIM], f32, kind="ExternalInput")
    b0cd = nc.dram_tensor("b0c", [H, 1], f32, kind="ExternalInput")
    b1rd = nc.dram_tensor("b1r", [1, H], f32, kind="ExternalInput")
    b2rd = nc.dram_tensor("b2r", [1, KDIM], f32, kind="ExternalInput")
    g0cd = nc.dram_tensor("g0c", [H, 1], f32, kind="ExternalInput")
    bt0cd = nc.dram_tensor("bt0c", [H, 1], f32, kind="ExternalInput")
    g1cd = nc.dram_tensor("g1c", [H, 1], f32, kind="ExternalInput")
    bt1cd = nc.dram_tensor("bt1c", [H, 1], f32, kind="ExternalInput")
    rsrd = nc.dram_tensor("rsr", [1, B], f32, kind="ExternalInput")
    invcd = nc.dram_tensor("invc", [B, 1], f32, kind="ExternalInput")
    outd = nc.dram_tensor("out", [B, KDIM], f32, kind="ExternalOutput")

    rg = [list(range(NCORES))]

    with tile.TileContext(nc) as tc:
        with (
            tc.tile_pool(name="const", bufs=1) as cpool,
            tc.tile_pool(name="xp", bufs=3) as xpool,
            tc.tile_pool(name="a1p", bufs=52) as a1pool,
            tc.tile_pool(name="zbp", bufs=3) as zbpool,
            tc.tile_pool(name="a2p", bufs=3) as a2pool,
            tc.tile_pool(name="misc", bufs=2) as mpool,
            tc.tile_pool(name="psA", bufs=2, space="PSUM") as psA,
            tc.tile_pool(name="psB", bufs=2, space="PSUM") as psB,
            tc.tile_pool(name="psG", bufs=1, space="PSUM") as psG,
            tc.tile_pool(name="psS", bufs=2, space="PSUM") as psS,
            tc.tile_pool(name="dram", bufs=1, space="DRAM") as dpool,
        ):
            # ---- constants / params in SBUF ----
            w0sb = cpool.tile([128, H], dt_c)  # W0 duplicated on both halves
            nc.sync.dma_start(w0sb[:], W0d[:])
            w1sb = cpool.tile([H, H], f32)
            nc.sync.dma_start(w1sb[:], W1d[:])
            w2sb = cpool.tile([H, KDIM], f32)
            nc.sync.dma_start(w2sb[:], W2d[:])
            b0c = cpool.tile([H, 1], f32)
            nc.sync.dma_start(b0c[:], b0cd[:])
            b1r = cpool.tile([1, H], f32)
            nc.sync.dma_start(b1r[:], b1rd[:])
            b2r = cpool.tile([1, KDIM], f32)
            nc.sync.dma_start(b2r[:], b2rd[:])
            g0c = cpool.tile([H, 1], f32)
            nc.sync.dma_start(g0c[:], g0cd[:])
            bt0c = cpool.tile([H, 1], f32)
            nc.sync.dma_start(bt0c[:], bt0cd[:])
            g1c = cpool.tile([H, 1], f32)
            nc.sync.dma_start(g1c[:], g1cd[:])
            bt1c = cpool.tile([H, 1], f32)
            nc.sync.dma_start(bt1c[:], bt1cd[:])
            rsr = cpool.tile([1, B], f32)
            nc.sync.dma_start(rsr[:], rsrd[:])
            invc = cpool.tile([B, 1], f32)
            nc.sync.dma_start(invc[:], invcd[:])

            ones_row = cpool.tile([1, H], f32)
            nc.vector.memset(ones_row[:], 1.0)
            ones_c = cpool.tile([1, H], dt_c)
            nc.vector.memset(ones_c[:], 1.0)
            id128 = cpool.tile([128, 128], f32)
            make_identity(nc, id128[:])

            # mask for the one subtile that straddles the real/pad boundary
            pad_frac = esh_real % SUB
            edge_mask = None
            if pad_frac:
                pidx = cpool.tile([128, 1], i32)
                nc.gpsimd.iota(pidx[:], pattern=[[0, 1]], base=0,
                               channel_multiplier=1)
                pidx_f = cpool.tile([128, 1], f32)
                nc.vector.tensor_copy(pidx_f[:], pidx[:])
                edge_mask = cpool.tile([128, 1], f32)
                nc.vector.tensor_scalar(
                    edge_mask[:], pidx_f[:], float(pad_frac), None,
                    op0=mybir.AluOpType.is_lt)

            stats1 = cpool.tile([H, 6 * NT], f32)

            # bei resident in SBUF, loaded during pass A
            bei_sb = cpool.tile([128, NSUB * BW], dt_c)
            for ch in range(NCH):
                lo = ch * (GATHER_BATCH // SUB) * BW
                hi = lo + (GATHER_BATCH // SUB) * BW
                nc.sync.dma_start(bei_sb[:, lo:hi], beiT[:, lo:hi])

            # ================= PASS A: BN1 stats =================
            for ch in range(NCH):
                xch = xpool.tile([128, CCOL], dt_c, tag="xch")
                nc.sync.dma_start(xch[:], xTi[:, ch * CCOL:(ch + 1) * CCOL])
                for tp in range(GATHER_BATCH // (2 * TILE)):
                    for u in range(2):
                        t = ch * (GATHER_BATCH // TILE) + 2 * tp + u
                        z1 = psA.tile([H, TILE], f32, space="PSUM", tag="z1")
                        nc.tensor.matmul(
                            z1[:], lhsT=w0sb[u * D:(u + 1) * D, :],
                            rhs=xch[u * D:(u + 1) * D, tp * TILE:(tp + 1) * TILE],
                            start=True, stop=True)
                        a1 = a1pool.tile([H, TILE], dt_c, tag="a1")
                        nc.scalar.activation(
                            a1[:], z1[:], mybir.ActivationFunctionType.Relu,
                            bias=b0c[:, 0:1])
                        pad_lo = esh_real - t * TILE
                        if pad_lo < TILE:
                            nc.vector.memset(a1[:, max(pad_lo, 0):TILE], 0.0)
                        nc.vector.bn_stats(stats1[:, 6 * t:6 * t + 6], a1[:])

            # ---- AllReduce #1: BN1 sums (issued before the prelude so the
            # collective overlaps with pass-B z1/a1 work) ----
            mv1 = mpool.tile([H, 2], f32, tag="mv")
            nc.vector.bn_aggr(mv1[:], stats1[:])
            # raw sums over this shard (pads are zero -> exact)
            ar1 = mpool.tile([H, 2], f32, tag="ar")
            nc.scalar.mul(ar1[:, 0:1], mv1[:, 0:1], float(ESH))
            msq1 = mpool.tile([H, 1], f32, tag="msq")
            nc.vector.tensor_mul(msq1[:], mv1[:, 0:1], mv1[:, 0:1])
            nc.vector.tensor_add(msq1[:], msq1[:], mv1[:, 1:2])
            nc.scalar.mul(ar1[:, 1:2], msq1[:], float(ESH))

            cc1_in = dpool.tile([H, 2], f32)
            cc1_out = dpool.tile([H, 2], f32)
            nc.sync.dma_start(cc1_in[:], ar1[:])
            nc.gpsimd.collective_compute(
                "AllReduce", mybir.AluOpType.add, replica_groups=rg,
                ins=[cc1_in.opt()], outs=[cc1_out.opt()])
            gs1 = mpool.tile([H, 2], f32, tag="gs")
            nc.sync.dma_start(gs1[:], cc1_out[:])

            # ---- prelude: z1/a1 for the first PRE tiles (AR1-independent) ----
            a1_keep = {}
            for ch in range(PRE // 4):
                xch = xpool.tile([128, CCOL], dt_c, tag="xch")
                nc.sync.dma_start(xch[:], xTi[:, ch * CCOL:(ch + 1) * CCOL])
                for tp in range(GATHER_BATCH // (2 * TILE)):
                    for u in range(2):
                        t = ch * (GATHER_BATCH // TILE) + 2 * tp + u
                        z1 = psA.tile([H, TILE], f32, space="PSUM", tag="z1")
                        nc.tensor.matmul(
                            z1[:], lhsT=w0sb[u * D:(u + 1) * D, :],
                            rhs=xch[u * D:(u + 1) * D, tp * TILE:(tp + 1) * TILE],
                            start=True, stop=True)
                        a1 = a1pool.tile([H, TILE], dt_c, tag="a1")
                        nc.vector.tensor_scalar(
                            a1[:], z1[:], b0c[:, 0:1], 0.0,
                            op0=mybir.AluOpType.add, op1=mybir.AluOpType.max)
                        a1_keep[t] = a1

            # mu, var, s1, t1
            mu1 = mpool.tile([H, 1], f32, tag="mu")
            nc.scalar.mul(mu1[:], gs1[:, 0:1], 1.0 / E_total)
            ex2 = mpool.tile([H, 1], f32, tag="ex2")
            nc.scalar.mul(ex2[:], gs1[:, 1:2], 1.0 / E_total)
            var1 = mpool.tile([H, 1], f32, tag="var")
            nc.vector.tensor_mul(var1[:], mu1[:], mu1[:])
            nc.vector.tensor_sub(var1[:], ex2[:], var1[:])
            sd1 = mpool.tile([H, 1], f32, tag="sd")
            nc.vector.tensor_scalar_add(sd1[:], var1[:], EPS)
            nc.scalar.sqrt(sd1[:], sd1[:])
            isd1 = mpool.tile([H, 1], f32, tag="isd")
            nc.vector.reciprocal(isd1[:], sd1[:])
            s1 = mpool.tile([H, 1], f32, tag="s1")
            nc.vector.tensor_mul(s1[:], g0c[:], isd1[:])
            t1 = mpool.tile([H, 1], f32, tag="t1")
            nc.vector.tensor_mul(t1[:], mu1[:], s1[:])
            nc.vector.tensor_sub(t1[:], bt0c[:], t1[:])

            # W1' (compute dtype), b1' tiled [1, TILE] (bias enters z2 PSUM
            # via a K=1 matmul)
            w1p = cpool.tile([H, H], dt_c)
            nc.vector.tensor_scalar_mul(w1p[:], w1sb[:], s1[:, 0:1])
            pr = psS.tile([1, H], f32, space="PSUM", tag="pss")
            nc.tensor.matmul(pr[:], lhsT=t1[:], rhs=w1sb[:], start=True, stop=True)
            b1tile = cpool.tile([1, TILE], dt_c)
            for s in range(NS):
                nc.vector.tensor_add(b1tile[:, s * H:(s + 1) * H], pr[:], b1r[:])

            # ============ PASS B: a2, G^T / sum2 / Gram accumulation ============
            gacc = psG.tile([H, GW], f32, space="PSUM", tag="gacc")
            for ch in range(NCH):
                if ch >= PRE // 4:
                    xch = xpool.tile([128, CCOL], dt_c, tag="xch")
                    nc.sync.dma_start(xch[:], xTi[:, ch * CCOL:(ch + 1) * CCOL])
                for tp in range(GATHER_BATCH // (2 * TILE)):
                    for u in range(2):
                        t = ch * (GATHER_BATCH // TILE) + 2 * tp + u
                        if t in a1_keep:
                            a1 = a1_keep.pop(t)
                        else:
                            z1 = psA.tile([H, TILE], f32, space="PSUM", tag="z1")
                            nc.tensor.matmul(
                                z1[:], lhsT=w0sb[u * D:(u + 1) * D, :],
                                rhs=xch[u * D:(u + 1) * D,
                                        tp * TILE:(tp + 1) * TILE],
                                start=True, stop=True)
                            a1 = a1pool.tile([H, TILE], dt_c, tag="a1")
                            nc.vector.tensor_scalar(
                                a1[:], z1[:], b0c[:, 0:1], 0.0,
                                op0=mybir.AluOpType.add, op1=mybir.AluOpType.max)
                        z2 = psB.tile([H, TILE], f32, space="PSUM", tag="z2")
                        nc.tensor.matmul(
                            z2[:], lhsT=ones_c[:], rhs=b1tile[:],
                            start=True, stop=False, skip_group_check=True)
                        for s in range(NS):
                            nc.tensor.matmul(
                                z2[:, s * H:(s + 1) * H],
                                lhsT=a1[:, s * SUB:(s + 1) * SUB],
                                rhs=w1p[:], start=False, stop=(s == NS - 1),
                                skip_group_check=True)
                        # a2t: per subtile [bei (32) | ones (1) | a2 (128)]
                        a2t = a2pool.tile([128, NS * GW], dt_c, tag="a2t")
                        a2t3 = a2t[:].rearrange("p (g c) -> p g c", c=GW)
                        nc.vector.tensor_copy(
                            a2t3[:, :, 0:BW],
                            bei_sb[:, (t * NS) * BW:(t * NS + NS) * BW]
                            .rearrange("p (g c) -> p g c", c=BW))
                        nc.scalar.activation(
                            a2t3[:, :, BW:GW],
                            z2[:].rearrange("p (g c) -> p g c", c=H),
                            mybir.ActivationFunctionType.Relu)
                        # zero a2 for pad edges (bei cols are host-zeroed)
                        for s in range(NS):
                            pl = esh_real - (t * NS + s) * SUB
                            if pl <= 0:
                                nc.vector.memset(a2t3[:, s, BW:GW], 0.0)
                            elif pl < SUB:
                                nc.vector.tensor_scalar_mul(
                                    a2t3[:, s, BW:GW], a2t3[:, s, BW:GW],
                                    edge_mask[:, 0:1])
                        first = (t == 0)
                        last = (t == NT - 1)
                        for s in range(NS):
                            nc.tensor.matmul(
                                gacc[:],
                                lhsT=a2t[:, s * GW + BW:(s + 1) * GW],
                                rhs=a2t[:, s * GW:(s + 1) * GW],
                                start=(first and s == 0),
                                stop=(last and s == NS - 1),
                                skip_group_check=True)

            # ---- AllReduce #2: [G^T | sum2 | sumsq2] ----
            garr = mpool.tile([H, BW + 1], f32, tag="garr")
            nc.vector.tensor_copy(garr[:, 0:BW], gacc[:, 0:BW])
            scr = mpool.tile([128, 128], f32, tag="scr")
            nc.vector.tensor_mul(scr[:], gacc[:, BW:GW], id128[:])
            nc.vector.tensor_reduce(
                garr[:, BW:BW + 1], scr[:], mybir.AxisListType.X,
                mybir.AluOpType.add)

            cc2_in = dpool.tile([H, BW + 1], f32)
            cc2_out = dpool.tile([H, BW + 1], f32)
            nc.sync.dma_start(cc2_in[:], garr[:])
            nc.gpsimd.collective_compute(
                "AllReduce", mybir.AluOpType.add, replica_groups=rg,
                ins=[cc2_in.opt()], outs=[cc2_out.opt()])
            gall = mpool.tile([H, BW + 1], f32, tag="gall")
            nc.sync.dma_start(gall[:], cc2_out[:])

            # ---- epilogue ----
            mu2 = mpool.tile([H, 1], f32, tag="mu")
            nc.scalar.mul(mu2[:], gall[:, B:B + 1], 1.0 / E_total)
            ex2b = mpool.tile([H, 1], f32, tag="ex2")
            nc.scalar.mul(ex2b[:], gall[:, BW:BW + 1], 1.0 / E_total)
            var2 = mpool.tile([H, 1], f32, tag="var")
            nc.vector.tensor_mul(var2[:], mu2[:], mu2[:])
            nc.vector.tensor_sub(var2[:], ex2b[:], var2[:])
            sd2 = mpool.tile([H, 1], f32, tag="sd")
            nc.vector.tensor_scalar_add(sd2[:], var2[:], EPS)
            nc.scalar.sqrt(sd2[:], sd2[:])
            isd2 = mpool.tile([H, 1], f32, tag="isd")
            nc.vector.reciprocal(isd2[:], sd2[:])
            s2 = mpool.tile([H, 1], f32, tag="s1")
            nc.vector.tensor_mul(s2[:], g1c[:], isd2[:])
            t2 = mpool.tile([H, 1], f32, tag="t1")
            nc.vector.tensor_mul(t2[:], mu2[:], s2[:])
            nc.vector.tensor_sub(t2[:], bt1c[:], t2[:])

            w2p = mpool.tile([H, KDIM], f32, tag="w2p")
            nc.vector.tensor_scalar_mul(w2p[:], w2sb[:], s2[:, 0:1])
            pr2 = psS.tile([1, KDIM], f32, space="PSUM", tag="pss")
            nc.tensor.matmul(pr2[:], lhsT=t2[:], rhs=w2sb[:], start=True, stop=True)
            b2p_row = mpool.tile([1, KDIM], f32, tag="b2pr")
            nc.vector.tensor_add(b2p_row[:], pr2[:], b2r[:])

            out_ps = psS.tile([B, KDIM], f32, space="PSUM", tag="pss")
            nc.tensor.matmul(out_ps[:], lhsT=gall[:, 0:B], rhs=w2p[:],
                             start=True, stop=False)
            nc.tensor.matmul(out_ps[:], lhsT=rsr[:], rhs=b2p_row[:],
                             start=False, stop=True)
            outsb = mpool.tile([B, KDIM], f32, tag="outsb")
            nc.vector.tensor_scalar_mul(outsb[:], out_ps[:], invc[:, 0:1])
            nc.sync.dma_start(outd[:], outsb[:])

    # Legalize waits for walrus (TRN2: max 1 wait/instruction; extras are
    # spilled onto ldweights / event-semaphore instructions).
    import bass_rust as _br
    _br.move_matmul_waits_to_ldweights(nc.m)
    _br.generate_event_semaphores(nc)
    nc.finalize()
    return nc


def _ceil_to(x, m):
    return (x + m - 1) // m * m


def make_inputs(inputs, ESH, N, dt_c=bf16, dt_en=None):
    """Host-side shard/layout prep. Returns in_maps for run_bass_kernel_spmd."""
    del N, dt_en
    np_c = _np_dt(dt_c)
    en = np.asarray(inputs["edge_nodes"], dtype=np.float32)
    x = np.asarray(inputs["edge_feats"], dtype=np.float32)
    src = np.asarray(inputs["src"]).astype(np.int64)
    dst = np.asarray(inputs["dst"]).astype(np.int64)
    E = x.shape[0]
    Nn = en.shape[1]
    esh_real = E // NCORES
    NSUB = ESH // SUB

    # exact row sums via degree counts (en entries are 0/1)
    deg = (np.bincount(src, minlength=Nn) + np.bincount(dst, minlength=Nn))
    rs = en.astype(np.float64) @ deg.astype(np.float64)
    inv = np.where(rs > 0, 1.0 / np.where(rs > 0, rs, 1.0), 0.0)

    enT = en.T  # [N, B]

    common = dict(
        W0=np.vstack([np.asarray(inputs["W0"], np.float32)] * 2).astype(np_c),
        W1=np.asarray(inputs["W1"], np.float32),
        W2=np.asarray(inputs["W2"], np.float32),
        b0c=np.asarray(inputs["b0"], np.float32).reshape(H, 1),
        b1r=np.asarray(inputs["b1"], np.float32).reshape(1, H),
        b2r=np.asarray(inputs["b2"], np.float32).reshape(1, KDIM),
        g0c=np.asarray(inputs["g0"], np.float32).reshape(H, 1),
        bt0c=np.asarray(inputs["bt0"], np.float32).reshape(H, 1),
        g1c=np.asarray(inputs["g1"], np.float32).reshape(H, 1),
        bt1c=np.asarray(inputs["bt1"], np.float32).reshape(H, 1),
        rsr=rs.astype(np.float32).reshape(1, B),
        invc=inv.astype(np.float32).reshape(B, 1),
    )

    in_maps = []
    for c in range(NCORES):
        lo = c * esh_real
        xs = x[lo:lo + esh_real]
        xT = np.zeros((D, ESH), np.float32)
        xT[:, :esh_real] = xs.T
        NTP = ESH // (2 * TILE)
        xTi = np.ascontiguousarray(
            xT.reshape(D, NTP, 2, TILE).transpose(2, 0, 1, 3).reshape(128, ESH // 2)
        ).astype(np_c)

        bei_aug = np.zeros((ESH, BW), np.float32)
        bei_aug[:esh_real, 0:B] = enT[src[lo:lo + esh_real]] + enT[dst[lo:lo + esh_real]]
        bei_aug[:esh_real, B] = 1.0
        beiT = np.ascontiguousarray(
            bei_aug.reshape(NSUB, 128, BW).transpose(1, 0, 2).reshape(128, NSUB * BW)
        ).astype(np_c)

        in_maps.append(dict(common, xTi=xTi, beiT=beiT))
    return in_maps


_NC_CACHE = {}


def kernel(**inputs):
    dt_c = bf16 if COMPUTE_DT == "bf16" else f32
    x = np.asarray(inputs["edge_feats"])
    en = np.asarray(inputs["edge_nodes"])
    E = x.shape[0]
    N = en.shape[1]
    ESH = _ceil_to(E // NCORES, GATHER_BATCH)
    key = (ESH, N, E, COMPUTE_DT)
    if key not in _NC_CACHE:
        _NC_CACHE[key] = build_nc(ESH, N, E, dt_c=dt_c)
    nc = _NC_CACHE[key]
    in_maps = make_inputs(inputs, ESH, N, dt_c=dt_c)
    res = run_bass_kernel_spmd(nc, in_maps, list(range(NCORES)))
    return np.asarray(res.results[0]["out"], np.float32)


# revision 22
# speedup vs baseline: 5.5610x; 1.5146x over previous
"""Trainium2 Bass kernel for DeepEdgeConvolution (gnn_message_passing).

Math (reference):
    bei = edge_nodes[:, src] + edge_nodes[:, dst]          # [B, E]
    bei = bei / row_sum (0 if empty row)
    h = BN1(relu(x @ W0 + b0)); h = BN2(relu(h @ W1 + b1)); h = h @ W2 + b2
    out = bei @ h                                          # [B, K]

Restructured: fold BN1 into (W1, b1) and BN2 into (W2, b2):
    a1 = relu(x @ W0 + b0)             (BN1 stats over E -> s1, t1)
    W1' = diag(s1) W1 ; b1' = t1 @ W1 + b1
    a2 = relu(a1 @ W1' + b1')          (BN2 stats over E -> s2, t2)
    W2' = diag(s2) W2 ; b2' = t2 @ W2 + b2
    out = diag(inv) [ (bei_raw @ a2) @ W2' + rs_raw x b2' ]

Sharding: edges across 8 cores; two streaming passes over x^T per core
(pass A: BN1 stats via bn_stats; pass B: recompute a1, then a2, G).

bei is computed on the HOST (sharding the columns of batch_edge_idcs per the
sharding hint): beiT is streamed as a dense packed input [128, NSUB*33] where
each 33-col block is [bei | 1] for one 128-edge subtile (edges on partitions).
Pad edges get all-zero columns (including the ones entry), which kills every
pad correction. Row sums rs / inv are exact small host-side inputs.

G accumulation (the [B,E]x[E,K] spmm): per 128-edge subtile,
    psG += a2_sub^T @ [bei_aug | a2_sub]    -> [H, 33 + H]
giving G^T (cols 0:32), sum(a2) (col 32, via the ones column) and the a2 Gram
matrix whose diagonal is sum(a2^2) -- one PSUM chain yields everything BN2 and
the final matmul need.  One AllReduce of [H, 34] follows; the epilogue is a
couple of tiny matmuls.
"""

import numpy as np

import concourse.bacc as bacc
import concourse.bass as bass
import concourse.tile as tile
from concourse import mybir
from concourse.bass_utils import run_bass_kernel_spmd
from concourse.masks import make_identity

f32 = mybir.dt.float32
bf16 = mybir.dt.bfloat16
i32 = mybir.dt.int32

NCORES = 8
B, D, H, KDIM = 32, 64, 128, 128
EPS = 1e-5
TILE = 512           # edges per tile
SUB = 128            # edges per matmul subtile
GATHER_BATCH = 2048  # edges per DMA chunk (ESH must be a multiple)

# compute dtype: "f32" (exact-ish) or "bf16" (fast).
COMPUTE_DT = "bf16"

BW = B + 1           # bei block width: [bei (32) | ones (1)]


def _np_dt(dt):
    if dt == bf16:
        import ml_dtypes
        return ml_dtypes.bfloat16
    return np.float32


def build_nc(ESH, N, E_total, dt_c=bf16, dt_en=None, debug=False):
    """Build the SPMD Bass program. ESH = padded edges per core."""
    del N, dt_en, debug
    assert ESH % GATHER_BATCH == 0
    NT = ESH // TILE          # tiles per core
    NSUB = ESH // SUB         # 128-edge subtiles per core
    NCH = ESH // GATHER_BATCH  # DMA chunks per core
    esh_real = E_total // NCORES
    assert E_total % NCORES == 0
    NS = TILE // SUB          # subtiles per tile (4)
    GW = BW + H               # gacc rhs width: [bei | 1 | a2]
    CCOL = GATHER_BATCH // 2  # packed x cols per chunk
    # pass-B head start: PREA a1 tiles retained from pass A, plus PREB tiles
    # whose z1/a1 is recomputed between pass A and the BN1 fold so the PE has
    # AllReduce-independent work while the collective is in flight.
    PREA = min(48, NT - NT % 4)
    PREB = min(32, NT - NT % 4 - PREA)

    nc = bass.Bass()

    # ---- I/O ----
    xTi = nc.dram_tensor("xTi", [128, ESH // 2], dt_c, kind="ExternalInput")
    beiT = nc.dram_tensor("beiT", [128, NSUB * BW], dt_c, kind="ExternalInput")
    W0d = nc.dram_tensor("W0", [2 * D, H], dt_c, kind="ExternalInput")
    W1d = nc.dram_tensor("W1", [H, H], f32, kind="ExternalInput")
    W2d = nc.dram_tensor("W2", [H, KDIM], f32, kind="ExternalInput")
    b0cd = nc.dram_tensor("b0c", [H, 1], f32, kind="ExternalInput")
    b1rd = nc.dram_tensor("b1r", [1, H], f32, kind="ExternalInput")
    b2rd = nc.dram_tensor("b2r", [1, KDIM], f32, kind="ExternalInput")
    g0cd = nc.dram_tensor("g0c", [H, 1], f32, kind="ExternalInput")
    bt0cd = nc.dram_tensor("bt0c", [H, 1], f32, kind="ExternalInput")
    g1cd = nc.dram_tensor("g1c", [H, 1], f32, kind="ExternalInput")
    bt1cd = nc.dram_tensor("bt1c", [H, 1], f32, kind="ExternalInput")
    rsrd = nc.dram_tensor("rsr", [1, B], f32, kind="ExternalInput")
    invcd = nc.dram_tensor("invc", [B, 1], f32, kind="ExternalInput")
    qcd = nc.dram_tensor("qc", [H, 1], f32, kind="ExternalInput")
    outd = nc.dram_tensor("out", [B, KDIM], f32, kind="ExternalOutput")

    rg = [list(range(NCORES))]

    with tile.TileContext(nc) as tc:
        with (
            tc.tile_pool(name="const", bufs=1) as cpool,
            tc.tile_pool(name="xp", bufs=3) as xpool,
            tc.tile_pool(name="a1p", bufs=84) as a1pool,
            tc.tile_pool(name="zbp", bufs=3) as zbpool,
            tc.tile_pool(name="a2p", bufs=3) as a2pool,
            tc.tile_pool(name="misc", bufs=2) as mpool,
            tc.tile_pool(name="psA", bufs=2, space="PSUM") as psA,
            tc.tile_pool(name="psB", bufs=2, space="PSUM") as psB,
            tc.tile_pool(name="psG", bufs=1, space="PSUM") as psG,
            tc.tile_pool(name="psS", bufs=2, space="PSUM") as psS,
            tc.tile_pool(name="dram", bufs=1, space="DRAM") as dpool,
        ):
            # ---- constants / params in SBUF ----
            w0sb = cpool.tile([128, H], dt_c)  # W0 duplicated on both halves
            nc.sync.dma_start(w0sb[:], W0d[:])
            w1sb = cpool.tile([H, H], f32)
            nc.sync.dma_start(w1sb[:], W1d[:])
            w2sb = cpool.tile([H, KDIM], f32)
            nc.sync.dma_start(w2sb[:], W2d[:])
            b0c = cpool.tile([H, 1], f32)
            nc.sync.dma_start(b0c[:], b0cd[:])
            b1r = cpool.tile([1, H], f32)
            nc.sync.dma_start(b1r[:], b1rd[:])
            b2r = cpool.tile([1, KDIM], f32)
            nc.sync.dma_start(b2r[:], b2rd[:])
            g0c = cpool.tile([H, 1], f32)
            nc.sync.dma_start(g0c[:], g0cd[:])
            bt0c = cpool.tile([H, 1], f32)
            nc.sync.dma_start(bt0c[:], bt0cd[:])
            g1c = cpool.tile([H, 1], f32)
            nc.sync.dma_start(g1c[:], g1cd[:])
            bt1c = cpool.tile([H, 1], f32)
            nc.sync.dma_start(bt1c[:], bt1cd[:])
            rsr = cpool.tile([1, B], f32)
            nc.sync.dma_start(rsr[:], rsrd[:])
            invc = cpool.tile([B, 1], f32)
            nc.sync.dma_start(invc[:], invcd[:])
            qc = cpool.tile([H, 1], f32)
            nc.sync.dma_start(qc[:], qcd[:])

            ones_row = cpool.tile([1, H], f32)
            nc.vector.memset(ones_row[:], 1.0)
            ones_c = cpool.tile([1, H], dt_c)
            nc.vector.memset(ones_c[:], 1.0)
            id128 = cpool.tile([128, 128], f32)
            make_identity(nc, id128[:])

            # mask for the one subtile that straddles the real/pad boundary
            pad_frac = esh_real % SUB
            edge_mask = None
            if pad_frac:
                pidx = cpool.tile([128, 1], i32)
                nc.gpsimd.iota(pidx[:], pattern=[[0, 1]], base=0,
                               channel_multiplier=1)
                pidx_f = cpool.tile([128, 1], f32)
                nc.vector.tensor_copy(pidx_f[:], pidx[:])
                edge_mask = cpool.tile([128, 1], f32)
                nc.vector.tensor_scalar(
                    edge_mask[:], pidx_f[:], float(pad_frac), None,
                    op0=mybir.AluOpType.is_lt)

            stats1 = cpool.tile([H, 6 * NT], f32)

            # bei resident in SBUF (DMA issues interleaved into pass A so the
            # SP issue queue doesn't delay the first x chunk)
            bei_sb = cpool.tile([128, NSUB * BW], dt_c)
            BCW = (GATHER_BATCH // SUB) * BW

            # ================= PASS A: BN1 stats =================
            # a1 of the first PREA tiles is retained for pass B.
            a1_keep = {}
            for ch in range(NCH):
                xch = xpool.tile([128, CCOL], dt_c, tag="xch")
                nc.sync.dma_start(xch[:], xTi[:, ch * CCOL:(ch + 1) * CCOL])
                nc.sync.dma_start(
                    bei_sb[:, ch * BCW:(ch + 1) * BCW],
                    beiT[:, ch * BCW:(ch + 1) * BCW])
                for tp in range(GATHER_BATCH // (2 * TILE)):
                    for u in range(2):
                        t = ch * (GATHER_BATCH // TILE) + 2 * tp + u
                        z1 = psA.tile([H, TILE], f32, space="PSUM", tag="z1")
                        nc.tensor.matmul(
                            z1[:], lhsT=w0sb[u * D:(u + 1) * D, :],
                            rhs=xch[u * D:(u + 1) * D, tp * TILE:(tp + 1) * TILE],
                            start=True, stop=True)
                        a1 = a1pool.tile([H, TILE], dt_c, tag="a1")
                        nc.scalar.activation(
                            a1[:], z1[:], mybir.ActivationFunctionType.Relu,
                            bias=b0c[:, 0:1])
                        pad_lo = esh_real - t * TILE
                        if pad_lo < TILE:
                            nc.vector.memset(a1[:, max(pad_lo, 0):TILE], 0.0)
                        nc.vector.bn_stats(stats1[:, 6 * t:6 * t + 6], a1[:])
                        if t < PREA:
                            a1_keep[t] = a1

            # ---- AllReduce #1: BN1 sums (issued before the prelude so the
            # collective overlaps with pass-B z1/a1 work) ----
            mv1 = mpool.tile([H, 2], f32, tag="mv")
            nc.vector.bn_aggr(mv1[:], stats1[:])
            # raw sums over this shard (pads are zero -> exact)
            ar1 = mpool.tile([H, 2], f32, tag="ar")
            nc.scalar.mul(ar1[:, 0:1], mv1[:, 0:1], float(ESH))
            msq1 = mpool.tile([H, 1], f32, tag="msq")
            nc.vector.tensor_mul(msq1[:], mv1[:, 0:1], mv1[:, 0:1])
            nc.vector.tensor_add(msq1[:], msq1[:], mv1[:, 1:2])
            nc.scalar.mul(ar1[:, 1:2], msq1[:], float(ESH))

            cc1_in = dpool.tile([H, 2], f32)
            cc1_out = dpool.tile([H, 2], f32)
            nc.sync.dma_start(cc1_in[:], ar1[:])
            nc.gpsimd.collective_compute(
                "AllReduce", mybir.AluOpType.add, replica_groups=rg,
                ins=[cc1_in.opt()], outs=[cc1_out.opt()])
            gs1 = mpool.tile([H, 2], f32, tag="gs")
            nc.sync.dma_start(gs1[:], cc1_out[:])

            # ---- prelude: z1/a1 of tiles [PREA, PREA+PREB) hides AR1 ----
            for ch in range(PREA // 4, (PREA + PREB) // 4):
                xch = xpool.tile([128, CCOL], dt_c, tag="xch")
                nc.sync.dma_start(xch[:], xTi[:, ch * CCOL:(ch + 1) * CCOL])
                for tp in range(GATHER_BATCH // (2 * TILE)):
                    for u in range(2):
                        t = ch * (GATHER_BATCH // TILE) + 2 * tp + u
                        z1 = psA.tile([H, TILE], f32, space="PSUM", tag="z1")
                        nc.tensor.matmul(
                            z1[:], lhsT=w0sb[u * D:(u + 1) * D, :],
                            rhs=xch[u * D:(u + 1) * D, tp * TILE:(tp + 1) * TILE],
                            start=True, stop=True)
                        a1 = a1pool.tile([H, TILE], dt_c, tag="a1")
                        nc.vector.tensor_scalar(
                            a1[:], z1[:], b0c[:, 0:1], 0.0,
                            op0=mybir.AluOpType.add, op1=mybir.AluOpType.max)
                        a1_keep[t] = a1

            # mu, var, s1, t1
            mu1 = mpool.tile([H, 1], f32, tag="mu")
            nc.scalar.mul(mu1[:], gs1[:, 0:1], 1.0 / E_total)
            ex2 = mpool.tile([H, 1], f32, tag="ex2")
            nc.scalar.mul(ex2[:], gs1[:, 1:2], 1.0 / E_total)
            var1 = mpool.tile([H, 1], f32, tag="var")
            nc.vector.tensor_mul(var1[:], mu1[:], mu1[:])
            nc.vector.tensor_sub(var1[:], ex2[:], var1[:])
            sd1 = mpool.tile([H, 1], f32, tag="sd")
            nc.vector.tensor_scalar_add(sd1[:], var1[:], EPS)
            nc.scalar.sqrt(sd1[:], sd1[:])
            isd1 = mpool.tile([H, 1], f32, tag="isd")
            nc.vector.reciprocal(isd1[:], sd1[:])
            s1 = mpool.tile([H, 1], f32, tag="s1")
            nc.vector.tensor_mul(s1[:], g0c[:], isd1[:])
            t1 = mpool.tile([H, 1], f32, tag="t1")
            nc.vector.tensor_mul(t1[:], mu1[:], s1[:])
            nc.vector.tensor_sub(t1[:], bt0c[:], t1[:])

            # W1' (compute dtype); bias b1' enters z2 through the a1 shift
            # delta = diag(1/s1)(t1 + W1^-T b1), since
            # (relu(z1+b0) + delta)^T W1' = a1^T W1' + b1'  and
            # relu(z1+b0) + delta = max(z1 + (b0+delta), delta).
            w1p = cpool.tile([H, H], dt_c)
            nc.vector.tensor_scalar_mul(w1p[:], w1sb[:], s1[:, 0:1])
            is1 = mpool.tile([H, 1], f32, tag="is1")
            nc.vector.reciprocal(is1[:], s1[:])
            delta = cpool.tile([H, 1], f32)
            nc.vector.tensor_add(delta[:], t1[:], qc[:])
            nc.vector.tensor_mul(delta[:], delta[:], is1[:])
            b0d = cpool.tile([H, 1], f32)
            nc.vector.tensor_add(b0d[:], b0c[:], delta[:])

            # ============ PASS B: a2, G^T / sum2 / Gram accumulation ============
            gacc = psG.tile([H, GW], f32, space="PSUM", tag="gacc")
            for ch in range(NCH):
                if ch >= (PREA + PREB) // 4:
                    xch = xpool.tile([128, CCOL], dt_c, tag="xch")
                    nc.sync.dma_start(xch[:], xTi[:, ch * CCOL:(ch + 1) * CCOL])
                for tp in range(GATHER_BATCH // (2 * TILE)):
                    for u in range(2):
                        t = ch * (GATHER_BATCH // TILE) + 2 * tp + u
                        if t in a1_keep:
                            a1 = a1_keep.pop(t)
                            nc.vector.tensor_scalar_add(
                                a1[:], a1[:], delta[:, 0:1])
                        else:
                            z1 = psA.tile([H, TILE], f32, space="PSUM", tag="z1")
                            nc.tensor.matmul(
                                z1[:], lhsT=w0sb[u * D:(u + 1) * D, :],
                                rhs=xch[u * D:(u + 1) * D,
                                        tp * TILE:(tp + 1) * TILE],
                                start=True, stop=True)
                            a1 = a1pool.tile([H, TILE], dt_c, tag="a1")
                            nc.vector.tensor_scalar(
                                a1[:], z1[:], b0d[:, 0:1], delta[:, 0:1],
                                op0=mybir.AluOpType.add, op1=mybir.AluOpType.max)
                        z2 = psB.tile([H, TILE], f32, space="PSUM", tag="z2")
                        for s in range(NS):
                            nc.tensor.matmul(
                                z2[:, s * H:(s + 1) * H],
                                lhsT=a1[:, s * SUB:(s + 1) * SUB],
                                rhs=w1p[:], start=True, stop=True)
                        # a2t: per subtile [bei (32) | ones (1) | a2 (128)]
                        a2t = a2pool.tile([128, NS * GW], dt_c, tag="a2t")
                        a2t3 = a2t[:].rearrange("p (g c) -> p g c", c=GW)
                        nc.vector.tensor_copy(
                            a2t3[:, :, 0:BW],
                            bei_sb[:, (t * NS) * BW:(t * NS + NS) * BW]
                            .rearrange("p (g c) -> p g c", c=BW))
                        nc.scalar.activation(
                            a2t3[:, :, BW:GW],
                            z2[:].rearrange("p (g c) -> p g c", c=H),
                            mybir.ActivationFunctionType.Relu)
                        # zero a2 for pad edges (bei cols are host-zeroed)
                        for s in range(NS):
                            pl = esh_real - (t * NS + s) * SUB
                            if pl <= 0:
                                nc.vector.memset(a2t3[:, s, BW:GW], 0.0)
                            elif pl < SUB:
                                nc.vector.tensor_scalar_mul(
                                    a2t3[:, s, BW:GW], a2t3[:, s, BW:GW],
                                    edge_mask[:, 0:1])
                        first = (t == 0)
                        last = (t == NT - 1)
                        for s in range(NS):
                            nc.tensor.matmul(
                                gacc[:],
                                lhsT=a2t[:, s * GW + BW:(s + 1) * GW],
                                rhs=a2t[:, s * GW:(s + 1) * GW],
                                start=(first and s == 0),
                                stop=(last and s == NS - 1),
                                skip_group_check=True)

            # ---- AllReduce #2: [G^T | sum2 | sumsq2] ----
            garr = mpool.tile([H, BW + 1], f32, tag="garr")
            nc.vector.tensor_copy(garr[:, 0:BW], gacc[:, 0:BW])
            scr = mpool.tile([128, 128], f32, tag="scr")
            nc.vector.tensor_mul(scr[:], gacc[:, BW:GW], id128[:])
            nc.vector.tensor_reduce(
                garr[:, BW:BW + 1], scr[:], mybir.AxisListType.X,
                mybir.AluOpType.add)

            cc2_in = dpool.tile([H, BW + 1], f32)
            cc2_out = dpool.tile([H, BW + 1], f32)
            nc.sync.dma_start(cc2_in[:], garr[:])
            nc.gpsimd.collective_compute(
                "AllReduce", mybir.AluOpType.add, replica_groups=rg,
                ins=[cc2_in.opt()], outs=[cc2_out.opt()])
            gall = mpool.tile([H, BW + 1], f32, tag="gall")
            nc.sync.dma_start(gall[:], cc2_out[:])

            # ---- epilogue ----
            mu2 = mpool.tile([H, 1], f32, tag="mu")
            nc.scalar.mul(mu2[:], gall[:, B:B + 1], 1.0 / E_total)
            ex2b = mpool.tile([H, 1], f32, tag="ex2")
            nc.scalar.mul(ex2b[:], gall[:, BW:BW + 1], 1.0 / E_total)
            var2 = mpool.tile([H, 1], f32, tag="var")
            nc.vector.tensor_mul(var2[:], mu2[:], mu2[:])
            nc.vector.tensor_sub(var2[:], ex2b[:], var2[:])
            sd2 = mpool.tile([H, 1], f32, tag="sd")
            nc.vector.tensor_scalar_add(sd2[:], var2[:], EPS)
            nc.scalar.sqrt(sd2[:], sd2[:])
            isd2 = mpool.tile([H, 1], f32, tag="isd")
            nc.vector.reciprocal(isd2[:], sd2[:])
            s2 = mpool.tile([H, 1], f32, tag="s1")
            nc.vector.tensor_mul(s2[:], g1c[:], isd2[:])
            t2 = mpool.tile([H, 1], f32, tag="t1")
            nc.vector.tensor_mul(t2[:], mu2[:], s2[:])
            nc.vector.tensor_sub(t2[:], bt1c[:], t2[:])

            w2p = mpool.tile([H, KDIM], f32, tag="w2p")
            nc.vector.tensor_scalar_mul(w2p[:], w2sb[:], s2[:, 0:1])
            pr2 = psS.tile([1, KDIM], f32, space="PSUM", tag="pss")
            nc.tensor.matmul(pr2[:], lhsT=t2[:], rhs=w2sb[:], start=True, stop=True)
            b2p_row = mpool.tile([1, KDIM], f32, tag="b2pr")
            nc.vector.tensor_add(b2p_row[:], pr2[:], b2r[:])

            out_ps = psS.tile([B, KDIM], f32, space="PSUM", tag="pss")
            nc.tensor.matmul(out_ps[:], lhsT=gall[:, 0:B], rhs=w2p[:],
                             start=True, stop=False)
            nc.tensor.matmul(out_ps[:], lhsT=rsr[:], rhs=b2p_row[:],
                             start=False, stop=True)
            outsb = mpool.tile([B, KDIM], f32, tag="outsb")
            nc.vector.tensor_scalar_mul(outsb[:], out_ps[:], invc[:, 0:1])
            nc.sync.dma_start(outd[:], outsb[:])

    # Legalize waits for walrus (TRN2: max 1 wait/instruction; extras are
    # spilled onto ldweights / event-semaphore instructions).
    import bass_rust as _br
    _br.move_matmul_waits_to_ldweights(nc.m)
    _br.generate_event_semaphores(nc)
    nc.finalize()
    return nc


def _ceil_to(x, m):
    return (x + m - 1) // m * m


def make_inputs(inputs, ESH, N, dt_c=bf16, dt_en=None):
    """Host-side shard/layout prep. Returns in_maps for run_bass_kernel_spmd."""
    del N, dt_en
    np_c = _np_dt(dt_c)
    en = np.asarray(inputs["edge_nodes"], dtype=np.float32)
    x = np.asarray(inputs["edge_feats"], dtype=np.float32)
    src = np.asarray(inputs["src"]).astype(np.int64)
    dst = np.asarray(inputs["dst"]).astype(np.int64)
    E = x.shape[0]
    Nn = en.shape[1]
    esh_real = E // NCORES
    NSUB = ESH // SUB

    # exact row sums via degree counts (en entries are 0/1)
    deg = (np.bincount(src, minlength=Nn) + np.bincount(dst, minlength=Nn))
    rs = en.astype(np.float64) @ deg.astype(np.float64)
    inv = np.where(rs > 0, 1.0 / np.where(rs > 0, rs, 1.0), 0.0)

    enT = en.T  # [N, B]

    common = dict(
        W0=np.vstack([np.asarray(inputs["W0"], np.float32)] * 2).astype(np_c),
        W1=np.asarray(inputs["W1"], np.float32),
        W2=np.asarray(inputs["W2"], np.float32),
        b0c=np.asarray(inputs["b0"], np.float32).reshape(H, 1),
        b1r=np.asarray(inputs["b1"], np.float32).reshape(1, H),
        b2r=np.asarray(inputs["b2"], np.float32).reshape(1, KDIM),
        g0c=np.asarray(inputs["g0"], np.float32).reshape(H, 1),
        bt0c=np.asarray(inputs["bt0"], np.float32).reshape(H, 1),
        g1c=np.asarray(inputs["g1"], np.float32).reshape(H, 1),
        bt1c=np.asarray(inputs["bt1"], np.float32).reshape(H, 1),
        rsr=rs.astype(np.float32).reshape(1, B),
        invc=inv.astype(np.float32).reshape(B, 1),
        qc=np.linalg.solve(
            np.asarray(inputs["W1"], np.float64).T,
            np.asarray(inputs["b1"], np.float64),
        ).astype(np.float32).reshape(H, 1),
    )

    in_maps = []
    for c in range(NCORES):
        lo = c * esh_real
        xs = x[lo:lo + esh_real]
        xT = np.zeros((D, ESH), np.float32)
        xT[:, :esh_real] = xs.T
        NTP = ESH // (2 * TILE)
        xTi = np.ascontiguousarray(
            xT.reshape(D, NTP, 2, TILE).transpose(2, 0, 1, 3).reshape(128, ESH // 2)
        ).astype(np_c)

        bei_aug = np.zeros((ESH, BW), np.float32)
        bei_aug[:esh_real, 0:B] = enT[src[lo:lo + esh_real]] + enT[dst[lo:lo + esh_real]]
        bei_aug[:esh_real, B] = 1.0
        beiT = np.ascontiguousarray(
            bei_aug.reshape(NSUB, 128, BW).transpose(1, 0, 2).reshape(128, NSUB * BW)
        ).astype(np_c)

        in_maps.append(dict(common, xTi=xTi, beiT=beiT))
    return in_maps


_NC_CACHE = {}


def kernel(**inputs):
    dt_c = bf16 if COMPUTE_DT == "bf16" else f32
    x = np.asarray(inputs["edge_feats"])
    en = np.asarray(inputs["edge_nodes"])
    E = x.shape[0]
    N = en.shape[1]
    ESH = _ceil_to(E // NCORES, GATHER_BATCH)
    key = (ESH, N, E, COMPUTE_DT)
    if key not in _NC_CACHE:
        _NC_CACHE[key] = build_nc(ESH, N, E, dt_c=dt_c)
    nc = _NC_CACHE[key]
    in_maps = make_inputs(inputs, ESH, N, dt_c=dt_c)
    res = run_bass_kernel_spmd(nc, in_maps, list(range(NCORES)))
    return np.asarray(res.results[0]["out"], np.float32)
